# revision 1
# baseline (speedup 1.0000x reference)
"""Trainium2 Bass kernel for nn_DinoGazeSpade (segment_reduce + SPADE stack).

Layout: 8 cores; image k = core//2; each core computes rows [16h, 16h+16) of
the 32x32 grid (h = core%2). Cross-core: 3 pairwise AllReduces of LayerNorm
partial stats. Heavy convs in fp16 matmuls, fp32 accumulate.

Key algebra:
  - painted map (448x448) never materialized: bilinear 448->32 samples exactly
    4 seg pixels per output at weight 1/4, so sm is the per-segment means
    avg[64,384] gathered through corner-count matrices; scatter_mean and gather
    are both matmuls against one-hot masks built via is_equal(iota, ids).
  - SPADE0's wb conv (128->1536) folded through conv0_w (1x1, 1536->8) on the
    host into a 128->8 conv. Same for SPADE1/2 wb convs.
  - LayerNorm linearized through the 1x1 convs: out = softplus(r*A + (-mu*r)*B
    + C + b) with A = W@(x .* gp1), B = W@gp1, C = fold(h) all independent of
    the stats, so every heavy matmul is emitted before any collective-dependent
    PE op (the PE queue is in-order; this hides the AllReduce latency).
  - LN stats via bn_stats/bn_aggr; rsqrt as exp(-0.5*ln(var+eps)) so every
    activation fits one ACT table set.
"""
import os
import numpy as np
from contextlib import ExitStack

import concourse.bass as bass
import concourse.mybir as mybir
import concourse.tile as tile
from concourse import bacc
from concourse.bass_utils import run_bass_kernel_spmd
from concourse.masks import make_identity

f32 = mybir.dt.float32
f16 = mybir.dt.float16
AF = mybir.ActivationFunctionType
ALU = mybir.AluOpType
AX = mybir.AxisListType

NSEG = 64
B, Cd, Hp, Wp, H, W, Cm, HID = 4, 384, 32, 32, 448, 448, 1536, 128
NPOS = Hp * Wp          # 1024
HROWS = 16              # rows per core
SMR = HROWS + 4         # sm rows incl 2-halo each side = 20
HR = HROWS + 2          # h rows incl 1-halo each side = 18
SMW = 34                # padded width

LAST_RESULTS = None  # set by kernel() for test harness introspection

_BUILT = None

TAPS = [(t // 3, t % 3) for t in range(9)]


def _softplus(nc, pool, z_in, bias_ap, out_tile, p, n, tag):
    """out = softplus(z_in + bias) = relu(z) + ln(1+exp(-|z|)) exactly."""
    t_abs = pool.tile([p, n], f32, tag="sp_abs", name=f"abs{tag}")
    nc.scalar.activation(out=t_abs, in_=z_in, func=AF.Abs, bias=bias_ap)
    t_exp = pool.tile([p, n], f32, tag="sp_exp", name=f"exp{tag}")
    nc.scalar.activation(out=t_exp, in_=t_abs, func=AF.Exp, scale=-1.0)
    t_ln = pool.tile([p, n], f32, tag="sp_ln", name=f"ln{tag}")
    nc.scalar.activation(out=t_ln, in_=t_exp, func=AF.Ln, bias=1.0)
    t_relu = pool.tile([p, n], f32, tag="sp_relu", name=f"relu{tag}")
    nc.scalar.activation(out=t_relu, in_=z_in, func=AF.Relu, bias=bias_ap)
    nc.vector.tensor_tensor(out=out_tile, in0=t_ln, in1=t_relu, op=ALU.add)


def _ln_finish(nc, pool, pst, work, n_inst, st_l, st_g, gid):
    """pst [2,1] = partial (sum of per-partition means, sum of E[x^2]).
    AllReduce over the pair -> r = 1/sqrt(var+eps), -mu*r in work[:, 5:7]."""
    st_sb = pool.tile([2, 1], f32, tag=f"st_sb{gid}", name=f"st_sb{gid}")
    nc.scalar.copy(st_sb, pst)
    nc.sync.dma_start(out=st_l[:], in_=st_sb[0:2, 0:1])
    nc.gpsimd.collective_compute(
        "AllReduce", ALU.add,
        replica_groups=[[0, 1], [2, 3], [4, 5], [6, 7]],
        ins=[st_l[:]], outs=[st_g[:]],
    )
    stg = pool.tile([1, 2], f32, tag=f"stg{gid}", name=f"stg{gid}")
    nc.sync.dma_start(out=stg, in_=st_g[None, :])
    nc.vector.tensor_scalar_mul(work[:, 0:2], stg[:, 0:2], 1.0 / n_inst)   # mu, E[x^2]
    nc.vector.tensor_tensor(out=work[:, 2:3], in0=work[:, 0:1], in1=work[:, 0:1], op=ALU.mult)
    nc.vector.tensor_tensor(out=work[:, 3:4], in0=work[:, 1:2], in1=work[:, 2:3], op=ALU.subtract)
    nc.scalar.activation(out=work[:, 4:5], in_=work[:, 3:4], func=AF.Ln, bias=1e-12)
    nc.scalar.activation(out=work[:, 5:6], in_=work[:, 4:5], func=AF.Exp, scale=-0.5)
    nc.vector.tensor_tensor(out=work[:, 7:8], in0=work[:, 0:1], in1=work[:, 5:6], op=ALU.mult)
    nc.vector.tensor_scalar_mul(work[:, 6:7], work[:, 7:8], -1.0)          # -mu*r


def _bn_partial(nc, pool, src, p, nchunks, tag):
    """bn_stats over src[p, nchunks, 512] -> mv[p,2] = (mean, E[x^2])."""
    bno = pool.tile([p, nchunks, 6], f32, tag=f"bno{tag}", name=f"bno{tag}")
    for kc in range(nchunks):
        nc.vector.bn_stats(out=bno[:, kc, :], in_=src[:, kc, :])
    mv = pool.tile([p, 2], f32, tag=f"mv{tag}", name=f"mv{tag}")
    nc.vector.bn_aggr(out=mv, in_=bno)
    m2 = pool.tile([p, 1], f32, tag=f"m2{tag}", name=f"m2{tag}")
    nc.vector.tensor_tensor(out=m2, in0=mv[:, 0:1], in1=mv[:, 0:1], op=ALU.mult)
    nc.vector.tensor_tensor(out=mv[:, 1:2], in0=mv[:, 1:2], in1=m2, op=ALU.add)
    return mv


def _build_nc():
    nc = bacc.Bacc("TRN2", num_devices=8)

    for val in (1e-12,):
        t = nc.alloc_sbuf_tensor(f"const-float32-{val}", [128, 1], f32)
        nc.gpsimd.memset(t.ap(), val)
        nc.const_aps.aps[(f32, val)] = t.ap()
    nc.all_engine_barrier()

    # ---------------- DRAM I/O ----------------
    d_x = nc.dram_tensor("x", [128, 12, 512], f16, kind="ExternalInput")
    d_ft = nc.dram_tensor("ft", [128, 8, 384], f16, kind="ExternalInput")
    d_ids = nc.dram_tensor("ids", [128, 8], f32, kind="ExternalInput")
    d_cid = nc.dram_tensor("cid", [128, 5, 4], f32, kind="ExternalInput")
    d_hmask = nc.dram_tensor("hmask", [HR], f16, kind="ExternalInput")
    d_ws = nc.dram_tensor("ws", [128, 3, 3, 9, 128], f16, kind="ExternalInput")
    d_wg = nc.dram_tensor("wg", [128, 12, 9, 128], f16, kind="ExternalInput")
    # wsm9 last-axis concat: wf0(8), wg1(8), wf1(16), wg2(16), wf2(1)
    d_wsm9 = nc.dram_tensor("wsm9", [128, 9, 49], f16, kind="ExternalInput")
    d_w0t = nc.dram_tensor("w0t", [128, 12, 8], f16, kind="ExternalInput")
    d_wsm = nc.dram_tensor("wsm", [144], f16, kind="ExternalInput")  # w1t|w2t
    d_bs = nc.dram_tensor("bs", [128, 3], f32, kind="ExternalInput")
    d_gb0 = nc.dram_tensor("gb0", [128, 12], f32, kind="ExternalInput")
    # biasv: gb1(8), gb2(16), b0f(8), b1f(16), b2f(1)
    d_biasv = nc.dram_tensor("biasv", [49], f32, kind="ExternalInput")
    d_out = nc.dram_tensor("out_half", [512], f32, kind="ExternalOutput")

    st_l = [nc.dram_tensor(f"st{i}_l", [2], f32) for i in range(3)]
    st_g = [nc.dram_tensor(f"st{i}_g", [2], f32) for i in range(3)]

    with ExitStack() as ctx:
        tc = ctx.enter_context(tile.TileContext(nc, num_cores=8))
        cpool = ctx.enter_context(tc.tile_pool(name="consts", bufs=1))
        dpool = ctx.enter_context(tc.tile_pool(name="data", bufs=1))
        spool = ctx.enter_context(tc.tile_pool(name="small", bufs=1))
        ps = ctx.enter_context(tc.tile_pool(name="ps", bufs=1, space="PSUM"))

        def MAIN(shape, name):
            return ps.tile(shape, f32, tag="ps_main", bufs=3, name=name)

        def ABC(name):
            return ps.tile([16, 512], f32, tag="ps_abc", bufs=3, name=name)

        def MISC(shape, dt, name):
            return ps.tile(shape, dt, tag="ps_misc", bufs=1, name=name)

        # ---- gpsimd constants first (iota gates the OH build) ----
        iot = cpool.tile([128, 64], f32)
        nc.gpsimd.iota(iot, pattern=[[1, 64]], base=0, channel_multiplier=0,
                       allow_small_or_imprecise_dtypes=True)
        ident = cpool.tile([128, 128], f16)
        make_identity(nc, ident)
        ones_col = cpool.tile([128, 1], f32)
        nc.gpsimd.memset(ones_col, 1.0)
        ones_row = cpool.tile([1, 128], f32)
        nc.gpsimd.memset(ones_row, 1.0)

        # --------- DMAs, ordered so early-needed data lands first ---------
        idst = cpool.tile([128, 8], f32)
        nc.sync.dma_start(out=idst, in_=d_ids[:, :])
        cidt = cpool.tile([128, 5, 4], f32)
        nc.sync.dma_start(out=cidt, in_=d_cid[:, :, :])
        feats = dpool.tile([128, 8, 385], f16)
        nc.sync.dma_start(out=feats[:, 0:4, 0:384], in_=d_ft[:, 0:4, :])
        nc.sync.dma_start(out=feats[:, 4:8, 0:384], in_=d_ft[:, 4:8, :])
        bs_t = cpool.tile([128, 3], f32)
        nc.sync.dma_start(out=bs_t, in_=d_bs[:, :])
        ws_t = cpool.tile([128, 3, 3, 9, 128], f16)
        nc.sync.dma_start(out=ws_t[:, 0:1], in_=d_ws[:, 0:1])     # s0_ws first
        gb0_t = cpool.tile([128, 12], f32)
        nc.sync.dma_start(out=gb0_t, in_=d_gb0[:, :])
        xt = dpool.tile([128, 12, 512], f16)
        nc.sync.dma_start(out=xt, in_=d_x[:, :, :])
        wg_t = cpool.tile([128, 12, 9, 128], f16)
        for g in range(3):
            nc.sync.dma_start(out=wg_t[:, g * 4:(g + 1) * 4], in_=d_wg[:, g * 4:(g + 1) * 4])
        nc.sync.dma_start(out=ws_t[:, 1:3], in_=d_ws[:, 1:3])     # s1/s2_ws
        w0t_t = cpool.tile([128, 12, 8], f16)
        nc.sync.dma_start(out=w0t_t, in_=d_w0t[:, :, :])
        wsm9_t = cpool.tile([128, 9, 49], f16)
        nc.sync.dma_start(out=wsm9_t, in_=d_wsm9[:, :, :])
        wf0_t = wsm9_t[:, :, 0:8]
        wg1_t = wsm9_t[:, :, 8:16]
        wf1_t = wsm9_t[:, :, 16:32]
        wg2_t = wsm9_t[:, :, 32:48]
        wf2_t = wsm9_t[:, :, 48:49]
        w1t_t = cpool.tile([8, 16], f16)
        nc.sync.dma_start(out=w1t_t, in_=d_wsm[0:128].rearrange("(a b) -> a b", b=16))
        w2t_t = cpool.tile([16, 1], f16)
        nc.sync.dma_start(out=w2t_t, in_=d_wsm[128:144][:, None])
        gb1_t = cpool.tile([8, 1], f32)
        nc.sync.dma_start(out=gb1_t, in_=d_biasv[0:8][:, None])
        gb2b = cpool.tile([16, 1], f32)
        nc.sync.dma_start(out=gb2b, in_=d_biasv[8:24][:, None])
        b0fb = cpool.tile([8, 1], f32)
        nc.sync.dma_start(out=b0fb, in_=d_biasv[24:32][:, None])
        b1fb = cpool.tile([16, 1], f32)
        nc.sync.dma_start(out=b1fb, in_=d_biasv[32:48][:, None])
        b2fb = cpool.tile([1, 1], f32)
        nc.sync.dma_start(out=b2fb, in_=d_biasv[48:49][:, None])
        hmask_bc = cpool.tile([128, HR], f16)
        nc.gpsimd.dma_start(out=hmask_bc, in_=d_hmask[None, :].to_broadcast([128, HR]))

        nc.gpsimd.memset(feats[:, :, 384:385], 1.0)
        # ---------------- segment means avg' [64, 384] ----------------
        oh_t = dpool.tile([128, 8, 64], f16)
        for qc in range(8):
            nc.vector.tensor_scalar(out=oh_t[:, qc, :], in0=iot,
                                    scalar1=idst[:, qc:qc + 1], scalar2=None,
                                    op0=ALU.is_equal)
        psums = ps.tile([64, 385], f32, tag="ps_sums", bufs=1)
        for qc in range(8):
            nc.tensor.matmul(psums, oh_t[:, qc, :], feats[:, qc, :],
                             start=(qc == 0), stop=(qc == 7))
        cnt4 = spool.tile([64, 1], f32, tag="cnt4")
        nc.vector.tensor_scalar(out=cnt4, in0=psums[:, 384:385], scalar1=1.0,
                                scalar2=4.0, op0=ALU.max, op1=ALU.mult)
        recip4 = spool.tile([64, 1], f32, tag="recip4")
        nc.vector.reciprocal(out=recip4, in_=cnt4)
        avg_t = dpool.tile([64, 384], f16)
        nc.vector.tensor_scalar_mul(avg_t, psums[:, 0:384], recip4[:, 0:1])

        # ---------------- G masks -> Gr [64, 640] ----------------
        gacc = dpool.tile([128, 5, 64], f16)
        gtmp = dpool.tile([128, 64], f16)
        for jc in range(5):
            nc.vector.tensor_scalar(out=gacc[:, jc, :], in0=iot,
                                    scalar1=cidt[:, jc, 0:1], scalar2=None,
                                    op0=ALU.is_equal)
            for corner in range(1, 4):
                nc.vector.tensor_scalar(out=gtmp, in0=iot,
                                        scalar1=cidt[:, jc, corner:corner + 1],
                                        scalar2=None, op0=ALU.is_equal)
                nc.vector.tensor_tensor(out=gacc[:, jc, :], in0=gacc[:, jc, :],
                                        in1=gtmp, op=ALU.add)
        gr_t = dpool.tile([64, 640], f16)
        for jc in range(5):
            ptr = MISC([64, 128], f16, f"ptr{jc}")
            nc.tensor.transpose(ptr, gacc[:, jc, :], ident)
            nc.scalar.copy(gr_t[:, jc * 128:(jc + 1) * 128], ptr)

        # ---------------- sm ----------------
        sm_pad = dpool.tile([128, 3, SMR, SMW], f16)
        nc.gpsimd.memset(sm_pad, 0.0)
        for mc in range(3):
            for nch in range(2):
                psm = MAIN([128, 320], f"psm{mc}{nch}")
                nc.tensor.matmul(psm, avg_t[:, mc * 128:(mc + 1) * 128],
                                 gr_t[:, nch * 320:(nch + 1) * 320],
                                 start=True, stop=True)
                nc.scalar.copy(sm_pad[:, mc, nch * 10:(nch + 1) * 10, 1:33],
                               psm.rearrange("p (r c) -> p r c", c=32))

        # ---------------- h conv helper ----------------
        def h_conv(cv):
            hp = dpool.tile([128, HR, SMW], f16, tag=f"hpad{cv}", name=f"hpad{cv}")
            nc.gpsimd.memset(hp, 0.0)
            for nch in range(2):
                psh = MAIN([128, 9 * 32], f"psh{cv}{nch}")
                for kc in range(3):
                    for t, (dy, dx) in enumerate(TAPS):
                        r0 = nch * 9 + dy
                        nc.tensor.matmul(
                            psh, ws_t[:, cv, kc, t, :],
                            sm_pad[:, kc, r0:r0 + 9, dx:dx + 32],
                            start=(kc == 0 and t == 0), stop=(kc == 2 and t == 8))
                nc.scalar.activation(
                    out=hp[:, nch * 9:(nch + 1) * 9, 1:33],
                    in_=psh.rearrange("p (r c) -> p r c", c=32),
                    func=AF.Relu, bias=bs_t[:, cv:cv + 1])
            nc.vector.tensor_tensor(
                out=hp, in0=hp,
                in1=hmask_bc[:, :, None].to_broadcast([128, HR, SMW]),
                op=ALU.mult)
            return hp

        h0p = h_conv(0)

        # ---------------- LN0 partial stats + collective (off PE path) ------
        mv0 = _bn_partial(nc, dpool, xt, 128, 12, "0")
        pst0 = MISC([2, 1], f32, "pst0")
        nc.tensor.matmul(pst0, mv0, ones_col, start=True, stop=True)
        work0 = spool.tile([1, 8], f32, tag="work0")
        _ln_finish(nc, spool, pst0, work0, 256.0, st_l[0], st_g[0], 0)

        # ---------------- conv_g + xg/gp1; A0/B0/C0 ----------------
        gp1 = dpool.tile([128, 12, 512], f16)
        xg = dpool.tile([128, 12, 512], f16)
        psA0 = ABC("psA0")
        psB0 = ABC("psB0")
        for kc in range(12):
            psg = MAIN([128, 512], f"psg{kc}")
            for t, (dy, dx) in enumerate(TAPS):
                nc.tensor.matmul(psg, wg_t[:, kc, t, :],
                                 h0p[:, dy:dy + 16, dx:dx + 32],
                                 start=(t == 0), stop=(t == 8))
            nc.scalar.activation(out=gp1[:, kc, :], in_=psg, func=AF.Identity,
                                 bias=gb0_t[:, kc:kc + 1])
            nc.vector.tensor_tensor(out=xg[:, kc, :], in0=xt[:, kc, :],
                                    in1=gp1[:, kc, :], op=ALU.mult)
        for kc in range(12):
            nc.tensor.matmul(psA0[0:8, :], w0t_t[:, kc, :], xg[:, kc, :],
                             start=(kc == 0), stop=(kc == 11))
        for kc in range(12):
            nc.tensor.matmul(psB0[0:8, :], w0t_t[:, kc, :], gp1[:, kc, :],
                             start=(kc == 0), stop=(kc == 11))
        psC0 = ABC("psC0")
        for t, (dy, dx) in enumerate(TAPS):
            nc.tensor.matmul(psC0[0:8, :], wf0_t[:, t, :],
                             h0p[:, dy:dy + 16, dx:dx + 32],
                             start=(t == 0), stop=(t == 8))

        # broadcast r0 / -mu0*r0 to 8 partitions (PE op, after CC0)
        pbc0 = MISC([8, 2], f32, "pbc0")
        nc.tensor.matmul(pbc0, ones_row[:, 0:8], work0[:, 5:7], start=True, stop=True)
        rbc0 = spool.tile([8, 2], f32, tag="rbc0")
        nc.scalar.copy(rbc0, pbc0)
        # z0 = r0*A0 + (-mu0*r0)*B0 + C0 ; out0 = softplus(z0 + b0f)
        z0 = dpool.tile([8, 512], f32, name="z0")
        zt0 = dpool.tile([8, 512], f32, name="zt0")
        nc.vector.tensor_scalar_mul(z0, psA0[0:8, :], rbc0[:, 0:1])
        nc.vector.tensor_scalar_mul(zt0, psB0[0:8, :], rbc0[:, 1:2])
        nc.vector.tensor_tensor(out=z0, in0=z0, in1=zt0, op=ALU.add)
        nc.vector.tensor_tensor(out=z0, in0=z0, in1=psC0[0:8, :], op=ALU.add)
        out0 = dpool.tile([8, 512], f32)
        _softplus(nc, dpool, z0, b0fb[:, 0:1], out0, 8, 512, "0")

        # ---------------- LN1 partial + collective ----------------
        mv1 = _bn_partial(nc, spool, out0[:, None, :], 8, 1, "1")
        pst1 = MISC([2, 1], f32, "pst1")
        nc.tensor.matmul(pst1, mv1, ones_col[0:8, :], start=True, stop=True)
        work1 = spool.tile([1, 8], f32, tag="work1")
        _ln_finish(nc, spool, pst1, work1, 16.0, st_l[1], st_g[1], 1)

        # PE work that fills the CC1 window
        h1p = h_conv(1)
        h2p = h_conv(2)
        psg1 = ABC("psg1")
        for t, (dy, dx) in enumerate(TAPS):
            nc.tensor.matmul(psg1[0:8, :], wg1_t[:, t, :],
                             h1p[:, dy:dy + 16, dx:dx + 32],
                             start=(t == 0), stop=(t == 8))
        gp11 = spool.tile([8, 512], f16, tag="gp11")
        nc.scalar.activation(out=gp11, in_=psg1[0:8, :], func=AF.Identity,
                             bias=gb1_t[:, 0:1])
        og1 = spool.tile([8, 512], f16, tag="og1")
        nc.vector.tensor_tensor(out=og1, in0=out0, in1=gp11, op=ALU.mult)
        psA1 = ABC("psA1")
        nc.tensor.matmul(psA1, w1t_t, og1, start=True, stop=True)
        psB1 = ABC("psB1")
        nc.tensor.matmul(psB1, w1t_t, gp11, start=True, stop=True)
        psC1 = ABC("psC1")
        for t, (dy, dx) in enumerate(TAPS):
            nc.tensor.matmul(psC1, wf1_t[:, t, :],
                             h1p[:, dy:dy + 16, dx:dx + 32],
                             start=(t == 0), stop=(t == 8))

        pbc1 = MISC([16, 2], f32, "pbc1")
        nc.tensor.matmul(pbc1, ones_row[:, 0:16], work1[:, 5:7], start=True, stop=True)
        rbc1 = spool.tile([16, 2], f32, tag="rbc1")
        nc.scalar.copy(rbc1, pbc1)
        z1 = dpool.tile([16, 512], f32, name="z1")
        zt1 = dpool.tile([16, 512], f32, name="zt1")
        nc.vector.tensor_scalar_mul(z1, psA1, rbc1[:, 0:1])
        nc.vector.tensor_scalar_mul(zt1, psB1, rbc1[:, 1:2])
        nc.vector.tensor_tensor(out=z1, in0=z1, in1=zt1, op=ALU.add)
        nc.vector.tensor_tensor(out=z1, in0=z1, in1=psC1, op=ALU.add)
        out1 = dpool.tile([16, 512], f32)
        _softplus(nc, dpool, z1, b1fb[:, 0:1], out1, 16, 512, "1")

        # ---------------- LN2 partial + collective ----------------
        mv2 = _bn_partial(nc, spool, out1[:, None, :], 16, 1, "2")
        pst2 = MISC([2, 1], f32, "pst2")
        nc.tensor.matmul(pst2, mv2, ones_col[0:16, :], start=True, stop=True)
        work2 = spool.tile([1, 8], f32, tag="work2")
        _ln_finish(nc, spool, pst2, work2, 32.0, st_l[2], st_g[2], 2)

        psg2 = ABC("psg2")
        for t, (dy, dx) in enumerate(TAPS):
            nc.tensor.matmul(psg2, wg2_t[:, t, :],
                             h2p[:, dy:dy + 16, dx:dx + 32],
                             start=(t == 0), stop=(t == 8))
        gp12 = spool.tile([16, 512], f16, tag="gp12")
        nc.scalar.activation(out=gp12, in_=psg2, func=AF.Identity,
                             bias=gb2b[:, 0:1])
        og2 = spool.tile([16, 512], f16, tag="og2")
        nc.vector.tensor_tensor(out=og2, in0=out1, in1=gp12, op=ALU.mult)
        psA2 = ABC("psA2")
        nc.tensor.matmul(psA2[0:1, :], w2t_t, og2, start=True, stop=True)
        psB2 = ABC("psB2")
        nc.tensor.matmul(psB2[0:1, :], w2t_t, gp12, start=True, stop=True)
        psC2 = ABC("psC2")
        for t, (dy, dx) in enumerate(TAPS):
            nc.tensor.matmul(psC2[0:1, :], wf2_t[:, t, :],
                             h2p[:, dy:dy + 16, dx:dx + 32],
                             start=(t == 0), stop=(t == 8))

        # final combine: scalars live on partition 0 -> no broadcast needed
        z2 = dpool.tile([1, 512], f32, name="z2")
        zt2 = dpool.tile([1, 512], f32, name="zt2")
        nc.vector.tensor_scalar_mul(z2, psA2[0:1, :], work2[:, 5:6])
        nc.vector.tensor_scalar_mul(zt2, psB2[0:1, :], work2[:, 6:7])
        nc.vector.tensor_tensor(out=z2, in0=z2, in1=zt2, op=ALU.add)
        nc.vector.tensor_tensor(out=z2, in0=z2, in1=psC2[0:1, :], op=ALU.add)
        final = dpool.tile([1, 512], f32)
        _softplus(nc, dpool, z2, b2fb[:, 0:1], final, 1, 512, "2")
        nc.sync.dma_start(out=d_out[:], in_=final[0:1, :])

    nc.compile()
    return nc


def _host_prep(inputs):
    """Build per-core in_maps (host work: slicing, layout, small weight folds)."""
    x_main = np.asarray(inputs["x_main"], np.float32)
    f_sem = np.asarray(inputs["f_sem"], np.float32)
    seg = np.asarray(inputs["seg_mask"])

    def lhsT9(w):  # [O, I, 3, 3] -> [I, 9, O]
        return np.ascontiguousarray(w.transpose(1, 2, 3, 0).reshape(w.shape[1], 9, w.shape[0]))

    ws_stack = np.stack([inputs["s0_ws"], inputs["s1_ws"], inputs["s2_ws"]])  # [3,128,384,3,3]
    ws_r = ws_stack.reshape(3, 128, 3, 128, 3, 3)          # cv, o, kc, i, ky, kx
    WS = np.ascontiguousarray(ws_r.transpose(3, 0, 2, 4, 5, 1)
                              .reshape(128, 3, 3, 9, 128)).astype(np.float16)
    wg0 = np.asarray(inputs["s0_wg"], np.float32)          # [1536, 128, 3, 3]
    WG = np.ascontiguousarray(
        wg0.reshape(12, 128, 128, 3, 3).transpose(2, 0, 3, 4, 1)
        .reshape(128, 12, 9, 128)).astype(np.float16)
    wf0 = np.einsum("oc,cikl->oikl", np.asarray(inputs["conv0_w"], np.float64),
                    np.asarray(inputs["s0_wb"], np.float64))
    wf1 = np.einsum("oc,cikl->oikl", np.asarray(inputs["conv1_w"], np.float64),
                    np.asarray(inputs["s1_wb"], np.float64))
    wf2 = np.einsum("oc,cikl->oikl", np.asarray(inputs["conv2_w"], np.float64),
                    np.asarray(inputs["s2_wb"], np.float64))
    WSM9 = np.concatenate([
        lhsT9(wf0), lhsT9(np.asarray(inputs["s1_wg"], np.float64)),
        lhsT9(wf1), lhsT9(np.asarray(inputs["s2_wg"], np.float64)),
        lhsT9(wf2)], axis=2).astype(np.float16)            # [128, 9, 49]
    W0T = np.ascontiguousarray(np.asarray(inputs["conv0_w"], np.float32).T
                               .reshape(12, 128, 8).transpose(1, 0, 2)).astype(np.float16)
    WSM = np.concatenate([
        np.asarray(inputs["conv1_w"], np.float32).T.reshape(-1),
        np.asarray(inputs["conv2_w"], np.float32).T.reshape(-1)]).astype(np.float16)  # [144]
    BS = np.ascontiguousarray(np.stack([inputs["s0_bs"], inputs["s1_bs"],
                                        inputs["s2_bs"]]).T).astype(np.float32)  # [128,3]
    GB0 = np.ascontiguousarray((1.0 + np.asarray(inputs["s0_bg"], np.float32))
                               .reshape(12, 128).T).astype(np.float32)           # [128,12]
    BIASV = np.concatenate([
        1.0 + np.asarray(inputs["s1_bg"], np.float64),
        1.0 + np.asarray(inputs["s2_bg"], np.float64),
        np.asarray(inputs["b0"], np.float64)
        + np.asarray(inputs["conv0_w"], np.float64) @ np.asarray(inputs["s0_bb"], np.float64),
        np.asarray(inputs["b1"], np.float64)
        + np.asarray(inputs["conv1_w"], np.float64) @ np.asarray(inputs["s1_bb"], np.float64),
        np.asarray(inputs["b2"], np.float64)
        + np.asarray(inputs["conv2_w"], np.float64) @ np.asarray(inputs["s2_bb"], np.float64),
    ]).astype(np.float32)                                   # [49]

    shared = dict(ws=WS, wg=WG, wsm9=WSM9, w0t=W0T, wsm=WSM, bs=BS, gb0=GB0,
                  biasv=BIASV)

    in_maps = []
    for core in range(8):
        k, h = core // 2, core % 2
        r0 = HROWS * h
        X = np.ascontiguousarray(
            x_main[k, :, r0:r0 + HROWS, :].reshape(12, 128, 512).transpose(1, 0, 2)
        ).astype(np.float16)
        FT = np.ascontiguousarray(
            f_sem[k].reshape(384, NPOS).T.reshape(8, 128, 384).transpose(1, 0, 2)
        ).astype(np.float16)
        ids_flat = seg[k, ::14, ::14].astype(np.float32).reshape(NPOS)
        IDS = np.ascontiguousarray(ids_flat.reshape(8, 128).T)
        rows = np.arange(r0 - 2, r0 + HROWS + 2)          # 20 sm rows
        valid = (rows >= 0) & (rows < Hp)
        rcl = np.clip(rows, 0, Hp - 1)
        cid = np.empty((SMR, Wp, 4), np.float32)
        cols = np.arange(Wp)
        for t, (dy, dx) in enumerate([(0, 0), (0, 1), (1, 0), (1, 1)]):
            v = seg[k][np.ix_(14 * rcl + 6 + dy, 14 * cols + 6 + dx)].astype(np.float32)
            v[~valid, :] = -1.0
            cid[:, :, t] = v
        CID = np.ascontiguousarray(cid.reshape(5, 128, 4).transpose(1, 0, 2))
        hrows = np.arange(r0 - 1, r0 + HROWS + 1)
        HM = ((hrows >= 0) & (hrows < Hp)).astype(np.float16)
        in_maps.append(dict(shared, x=X, ft=FT, ids=IDS, cid=CID, hmask=HM))
    return in_maps


def kernel(**inputs):
    global _BUILT, LAST_RESULTS
    if _BUILT is None:
        _BUILT = _build_nc()
    nc = _BUILT
    in_maps = _host_prep(inputs)
    trace = bool(os.environ.get("BASS_TRACE"))
    res = run_bass_kernel_spmd(nc, in_maps, list(range(8)), trace=trace)
    LAST_RESULTS = res
    out = np.empty((B, 1, Hp, Wp), np.float32)
    for core in range(8):
        k, h = core // 2, core % 2
        out[k, 0, HROWS * h:HROWS * (h + 1), :] = \
            res.results[core]["out_half"].reshape(HROWS, Wp)
    return out



# revision 30
# speedup vs baseline: 1.0314x; 1.0314x over previous
"""Trainium2 Bass kernel for nn_DinoGazeSpade (segment_reduce + SPADE stack).

Layout: 8 cores; image k = core//2; each core computes rows [16h, 16h+16) of
the 32x32 grid (h = core%2). Cross-core: 3 pairwise AllReduces of LayerNorm
partial stats.

Structure (v2):
  - All 3x3 convs (sm->h, h->gp1, h->C/B folds, h->gp11/gp12) run as fp8e4m3
    DoubleRow matmuls (2 k-tiles per op) with host power-of-2 weight scaling;
    descale folded into the PSUM-read activation / combine scalars.
  - B terms (W @ gp) folded on host into conv weights from h (W@wg), merged
    into the same PSUM group as the C fold -> no gp-dependent projections.
  - LayerNorm stats never touch the PE: bn_stats/aggr (vector) ->
    gpsimd tensor_reduce(axis=C) -> DMA -> AllReduce -> broadcast-DMA of the
    raw sums to all partitions; rsqrt = vector.reciprocal + scalar Sqrt.
  - Scalar engine uses only {Sqrt, Relu, Identity, Copy, Square} => a single
    activation table set, no ACT_TABLE_LOAD ping-pong. softplus = relu(z) +
    rational(2,3)(min(|z|,8)) on the vector engine (max abs err ~7e-4).
  - PE program order: all stat-independent matmuls first (oh, sm, h0..h2,
    C/B folds, gp convs, conv_g+A0 pipelined), A1/A2 at the tail.
  - h-conv halo rows are zeroed by masking the PSUM (f32) before the relu.
"""
import os
import numpy as np
from contextlib import ExitStack

import ml_dtypes

import concourse.bass as bass
import concourse.mybir as mybir
import concourse.tile as tile
from concourse import bacc
from concourse.bass_utils import run_bass_kernel_spmd
from concourse.masks import make_identity

f32 = mybir.dt.float32
f16 = mybir.dt.float16
f8 = mybir.dt.float8e4
AF = mybir.ActivationFunctionType
ALU = mybir.AluOpType
AX = mybir.AxisListType
DR = mybir.MatmulPerfMode.DoubleRow

NSEG = 64
B, Cd, Hp, Wp, H, W, Cm, HID = 4, 384, 32, 32, 448, 448, 1536, 128
NPOS = Hp * Wp          # 1024
HROWS = 16              # rows per core
SMR = HROWS + 4         # sm rows incl 2-halo each side = 20
HR = HROWS + 2          # h rows incl 1-halo each side = 18
SMW = 48                # padded width (row stride must be 16B-aligned for dual-fp8)

# fp8 scale exponents (power of two)
S_SM = 8.0      # sm values
S_WS = 64.0     # ws weights
S_H = 4.0       # h values
S_WG = 64.0     # conv_g / wg1 / wg2 weights
S_F0 = 64.0     # layer-0 C/B fold weights
S_F12 = 256.0   # layer-1/2 C/B fold weights

# softplus tail g(t)=ln(1+exp(-t)) ~ (c0+c1 t+c2 t^2)/(1+d1 t+d2 t^2+d3 t^3)
# fit on t in [0,8] (t clamped at 8; g(8)=3.35e-4), max abs err 3.9e-4
SP_C0, SP_C1, SP_C2 = 0.6934867715618367, -0.17760652420286008, 0.011840728429853564
SP_D1, SP_D2, SP_D3 = 0.477190455932838, 0.1387482411055944, 0.0669674223997194

LAST_RESULTS = None  # set by kernel() for test harness introspection

_BUILT = None

TAPS = [(t // 3, t % 3) for t in range(9)]
# Dual-fp8 moving APs need all outer steps 16B-aligned: pair taps VERTICALLY
# (delta = row stride 48). Pairs (t, t+3) for t in 0..2; taps 6..8 single.
# Weight tensors store taps in order [0,3,1,4,2,5,6,7,8] so pairs and the
# k-tile dim are adjacent.
VPAIRS = [(0, 3), (1, 4), (2, 5)]
VSINGLES = [6, 7, 8]
TAP_ORDER = [0, 3, 1, 4, 2, 5, 6, 7, 8]


def _pair_ap(a, delta):
    """Insert a [delta, 2] k-tile dim as dim 1 of an AP (for DoubleRow rhs)."""
    ap = list(a.ap)
    new = [ap[0], [delta, 2]] + list(ap[1:])
    return bass.AP(a.tensor, a.offset, new)


def _softplus(nc, pool, z, out_tile, p, tag):
    """out = softplus(z) = relu(z) + g(min(|z|,8)); scalar does the relu."""
    ta = pool.tile([p, 512], f32, tag=f"sp_ta{tag}")
    nc.scalar.activation(out=ta, in_=z, func=AF.Abs, bias=0.0)
    t = pool.tile([p, 512], f32, tag=f"sp_t{tag}")
    nc.vector.tensor_scalar(out=t, in0=ta, scalar1=8.0, scalar2=None,
                            op0=ALU.min)
    t2 = pool.tile([p, 512], f32, tag=f"sp_t2{tag}")
    nc.vector.tensor_tensor(out=t2, in0=t, in1=t, op=ALU.mult)
    rl = pool.tile([p, 512], f32, tag=f"sp_rl{tag}")
    nc.scalar.activation(out=rl, in_=z, func=AF.Relu, bias=0.0)
    n1 = pool.tile([p, 512], f32, tag=f"sp_n1{tag}")
    nc.vector.tensor_scalar(out=n1, in0=t, scalar1=SP_C1, scalar2=SP_C0,
                            op0=ALU.mult, op1=ALU.add)
    num = pool.tile([p, 512], f32, tag=f"sp_num{tag}")
    nc.vector.scalar_tensor_tensor(out=num, in0=t2, scalar=SP_C2, in1=n1,
                                   op0=ALU.mult, op1=ALU.add)
    d1p = pool.tile([p, 512], f32, tag=f"sp_d1{tag}")
    nc.vector.tensor_scalar(out=d1p, in0=t, scalar1=SP_D1, scalar2=1.0,
                            op0=ALU.mult, op1=ALU.add)
    q = pool.tile([p, 512], f32, tag=f"sp_q{tag}")
    nc.vector.tensor_scalar(out=q, in0=t, scalar1=SP_D3, scalar2=SP_D2,
                            op0=ALU.mult, op1=ALU.add)
    den = pool.tile([p, 512], f32, tag=f"sp_den{tag}")
    nc.vector.tensor_tensor(out=den, in0=q, in1=t2, op=ALU.mult)
    nc.vector.tensor_tensor(out=den, in0=den, in1=d1p, op=ALU.add)
    rd = pool.tile([p, 512], f32, tag=f"sp_rd{tag}")
    nc.vector.reciprocal(out=rd, in_=den)
    gg = pool.tile([p, 512], f32, tag=f"sp_gg{tag}")
    nc.vector.tensor_tensor(out=gg, in0=num, in1=rd, op=ALU.mult)
    nc.vector.tensor_tensor(out=out_tile, in0=gg, in1=rl, op=ALU.add)


def _build_nc():
    stage = int(os.environ.get("KBISECT", "99"))
    nc = bacc.Bacc("TRN2", num_devices=8)

    # (f32, 0.0) const AP is pre-registered by the framework
    nc.all_engine_barrier()

    # ---------------- DRAM I/O ----------------
    d_x = nc.dram_tensor("x", [128, 12, 512], f16, kind="ExternalInput")
    d_ft = nc.dram_tensor("ft", [128, 8, 384], f16, kind="ExternalInput")
    d_ids = nc.dram_tensor("ids", [128, 8], f32, kind="ExternalInput")
    d_cid = nc.dram_tensor("cid", [128, 5, 4], f32, kind="ExternalInput")
    d_hmask = nc.dram_tensor("hmask", [HR], f32, kind="ExternalInput")
    d_ws01 = nc.dram_tensor("ws01", [128, 3, 9, 2, 128], f8, kind="ExternalInput")
    d_ws2 = nc.dram_tensor("ws2", [128, 3, 9, 128], f8, kind="ExternalInput")
    d_wg = nc.dram_tensor("wg", [128, 12, 9, 128], f8, kind="ExternalInput")
    # wsm9: 8 groups of 64 cols (dual-fp8 w/ windowed moving wants >=64):
    # wf0@0(8) wb0@64(8) wg1@128(8) wf1@192(16) wb1@256(16) wg2@320(16)
    # wf2@384(1) wb2@448(1)
    d_wsm9 = nc.dram_tensor("wsm9", [128, 9, 512], f8, kind="ExternalInput")
    d_w0t = nc.dram_tensor("w0t", [128, 12, 8], f16, kind="ExternalInput")
    d_wsm = nc.dram_tensor("wsm", [144], f16, kind="ExternalInput")  # w1t|w2t
    d_bsgb = nc.dram_tensor("bsgb", [128, 15], f32, kind="ExternalInput")
    # biasv [16,8] cols: gb1 gb2 b0f b1f b2f B0c B1c B2c (each from row 0)
    d_biasv = nc.dram_tensor("biasv", [16, 8], f32, kind="ExternalInput")
    d_out = nc.dram_tensor("out_half", [512], f32, kind="ExternalOutput")

    st_l = [nc.dram_tensor(f"st{i}_l", [2], f32) for i in range(3)]
    st_g = [nc.dram_tensor(f"st{i}_g", [2], f32) for i in range(3)]

    with ExitStack() as ctx:
        tc = ctx.enter_context(tile.TileContext(nc, num_cores=8))
        cpool = ctx.enter_context(tc.tile_pool(name="consts", bufs=1))
        dpool = ctx.enter_context(tc.tile_pool(name="data", bufs=1))
        spool = ctx.enter_context(tc.tile_pool(name="small", bufs=1))
        ps = ctx.enter_context(tc.tile_pool(name="ps", bufs=1, space="PSUM"))

        def MAIN(shape, name):
            return ps.tile(shape, f32, tag="ps_main", bufs=2, name=name)

        def PSA(shape, name):  # psA0 -> psA1 -> psA2
            return ps.tile(shape, f32, tag="ps_a", bufs=1, name=name)

        def PSC(shape, name):  # psC0 -> psC1 -> psC2
            return ps.tile(shape, f32, tag="ps_c", bufs=2, name=name)

        def PSB(shape, name):  # seg sums -> psB0 -> psB1 -> psB2 (bank reuse)
            return ps.tile(shape, f32, tag="ps_sums", bufs=1, name=name)

        def PSG(shape, name, dt=f32):  # gr transposes, psg1, psg2
            return ps.tile(shape, dt, tag="ps_g", bufs=2, name=name)

        # ---- gpsimd constants first (iota gates the OH build) ----
        iot = cpool.tile([128, 64], f32)
        nc.gpsimd.iota(iot, pattern=[[1, 64]], base=0, channel_multiplier=0,
                       allow_small_or_imprecise_dtypes=True)
        ident = cpool.tile([128, 128], f16)
        make_identity(nc, ident)
        ones_col = cpool.tile([128, 1], f32)
        nc.gpsimd.memset(ones_col, 1.0)

        # --------- DMAs: sync queue = big early tensors, in need order ------
        idst = cpool.tile([128, 8], f32)
        nc.sync.dma_start(out=idst, in_=d_ids[:, :])
        cidt = cpool.tile([128, 5, 4], f32)
        nc.sync.dma_start(out=cidt, in_=d_cid[:, :, :])
        feats = dpool.tile([128, 8, 385], f16)
        nc.sync.dma_start(out=feats[:, 0:4, 0:384], in_=d_ft[:, 0:4, :])
        nc.sync.dma_start(out=feats[:, 4:8, 0:384], in_=d_ft[:, 4:8, :])
        ws01_t = cpool.tile([128, 3, 9, 2, 128], f8)
        nc.sync.dma_start(out=ws01_t[:, 0:1], in_=d_ws01[:, 0:1])  # s0_ws first
        ws2_t = cpool.tile([128, 3, 9, 128], f8)
        nc.sync.dma_start(out=ws2_t[:, 0:1], in_=d_ws2[:, 0:1])
        xt = dpool.tile([128, 12, 512], f16)
        nc.sync.dma_start(out=xt, in_=d_x[:, :, :])
        wg_t = cpool.tile([128, 12, 9, 128], f8)
        for g in range(3):
            nc.sync.dma_start(out=wg_t[:, g * 4:(g + 1) * 4],
                              in_=d_wg[:, g * 4:(g + 1) * 4])
        nc.sync.dma_start(out=ws01_t[:, 1:3], in_=d_ws01[:, 1:3])  # s1/s2_ws
        nc.sync.dma_start(out=ws2_t[:, 1:3], in_=d_ws2[:, 1:3])

        # --------- small/later tensors issued from the scalar queue ---------
        wsm9_t = cpool.tile([128, 9, 512], f8)
        nc.scalar.dma_start(out=wsm9_t, in_=d_wsm9[:, :, :])
        w0t_t = cpool.tile([128, 12, 8], f16)
        nc.scalar.dma_start(out=w0t_t, in_=d_w0t[:, :, :])
        bsgb_t = cpool.tile([128, 15], f32)
        nc.scalar.dma_start(out=bsgb_t, in_=d_bsgb[:, :])
        bs_t = bsgb_t[:, 0:3]          # S_H * bs, per conv
        gb0_t = bsgb_t[:, 3:15]        # 1 + bg0
        w1t_t = cpool.tile([8, 16], f16)
        nc.scalar.dma_start(out=w1t_t, in_=d_wsm[0:128].rearrange("(a b) -> a b", b=16))
        w2t_t = cpool.tile([16, 1], f16)
        nc.scalar.dma_start(out=w2t_t, in_=d_wsm[128:144][:, None])
        biasv_t = cpool.tile([16, 8], f32)
        nc.scalar.dma_start(out=biasv_t, in_=d_biasv[:, :])
        gb1_t = biasv_t[0:8, 0:1]
        gb2_t = biasv_t[0:16, 1:2]
        b0f_t = biasv_t[0:8, 2:3]
        b1f_t = biasv_t[0:16, 3:4]
        b2f_t = biasv_t[0:1, 4:5]
        B0c_t = biasv_t[0:8, 5:6]
        B1c_t = biasv_t[0:16, 6:7]
        B2c_t = biasv_t[0:1, 7:8]
        hmask_bc = cpool.tile([128, HR], f32)
        nc.scalar.dma_start(out=hmask_bc, in_=d_hmask[None, :].to_broadcast([128, HR]))

        # ---- engine warmups during the DMA window ----
        warm = cpool.tile([128, 64], f32)
        for _ in range(3):
            nc.vector.memset(warm, 0.0)
        pswarm = MAIN([128, 128], "pswarm")
        for _ in range(6):
            nc.tensor.matmul(pswarm, ident, ident, start=True, stop=True)
        wread = cpool.tile([128, 128], f32)
        nc.scalar.copy(wread, pswarm)

        nc.gpsimd.memset(feats[:, :, 384:385], 1.0)

        # ---------------- segment one-hots (vector) ----------------
        oh_t = dpool.tile([128, 8, 64], f16)
        for qc in range(8):
            nc.vector.tensor_scalar(out=oh_t[:, qc, :], in0=iot,
                                    scalar1=idst[:, qc:qc + 1], scalar2=None,
                                    op0=ALU.is_equal)
        gacc = dpool.tile([128, 5, 64], f16)
        gtmp = dpool.tile([128, 64], f16)
        for jc in range(5):
            nc.vector.tensor_scalar(out=gacc[:, jc, :], in0=iot,
                                    scalar1=cidt[:, jc, 0:1], scalar2=None,
                                    op0=ALU.is_equal)
            for corner in range(1, 4):
                nc.vector.tensor_scalar(out=gtmp, in0=iot,
                                        scalar1=cidt[:, jc, corner:corner + 1],
                                        scalar2=None, op0=ALU.is_equal)
                nc.vector.tensor_tensor(out=gacc[:, jc, :], in0=gacc[:, jc, :],
                                        in1=gtmp, op=ALU.add)

        # ---------------- segment means avg' [64, 384] ----------------
        psums = ps.tile([64, 385], f32, tag="ps_sums", bufs=1)
        for qc in range(8):
            nc.tensor.matmul(psums, oh_t[:, qc, :], feats[:, qc, :],
                             start=(qc == 0), stop=(qc == 7))
        cnt4 = spool.tile([64, 1], f32, tag="cnt4")
        nc.vector.tensor_scalar(out=cnt4, in0=psums[:, 384:385], scalar1=1.0,
                                scalar2=4.0, op0=ALU.max, op1=ALU.mult)
        recip4 = spool.tile([64, 1], f32, tag="recip4")
        nc.vector.reciprocal(out=recip4, in_=cnt4)
        avg_t = dpool.tile([64, 384], f16)
        nc.vector.tensor_scalar_mul(avg_t, psums[:, 0:384], recip4[:, 0:1])

        # ---------------- G masks -> Gr [64, 640] ----------------
        gr_t = dpool.tile([64, 640], f16)
        for jc in range(5):
            ptr = PSG([64, 128], f"ptr{jc}", dt=f16)
            nc.tensor.transpose(ptr, gacc[:, jc, :], ident)
            nc.scalar.copy(gr_t[:, jc * 128:(jc + 1) * 128], ptr)

        # ---------------- sm (f8, scaled by S_SM) ----------------
        sm_pad = dpool.tile([128, 3, SMR, SMW], f8)
        nc.gpsimd.memset(sm_pad, 0.0)
        for mc in range(3):
            for nch in range(2):
                psm = MAIN([128, 320], f"psm{mc}{nch}")
                nc.tensor.matmul(psm, avg_t[:, mc * 128:(mc + 1) * 128],
                                 gr_t[:, nch * 320:(nch + 1) * 320],
                                 start=True, stop=True)
                nc.scalar.activation(
                    out=sm_pad[:, mc, nch * 10:(nch + 1) * 10, 1:33],
                    in_=psm.rearrange("p (r c) -> p r c", c=32),
                    func=AF.Copy, scale=S_SM)

        # ---------------- h convs (fp8 DoubleRow over kc pairs + taps) ------
        hps = []

        def h_conv(cv):
            hp = dpool.tile([128, HR, SMW], f8, tag=f"hpad{cv}", name=f"hpad{cv}")
            nc.gpsimd.memset(hp, 0.0)
            for nch in range(2):
                psh = MAIN([128, 9 * 32], f"psh{cv}{nch}")
                psh3 = psh.rearrange("p (r c) -> p r c", c=32)
                # kc-pair (0,1) DoubleRow per tap (k-tile delta = plane stride)
                for t, (dy, dx) in enumerate(TAPS):
                    r0 = nch * 9 + dy
                    mv = sm_pad[:, 0, r0:r0 + 9, dx:dx + 32]
                    mv2 = _pair_ap(mv, SMR * SMW)
                    nc.tensor.matmul(psh, ws01_t[:, cv, t, :, :], mv2,
                                     start=(t == 0), stop=False, perf_mode=DR)
                # kc=2: vertical tap-pairs (delta = row stride) + 3 singles
                for i, (ta, tb) in enumerate(VPAIRS):
                    dy, dx = TAPS[ta]
                    r0 = nch * 9 + dy
                    mv = sm_pad[:, 2, r0:r0 + 9, dx:dx + 32]
                    mv2 = _pair_ap(mv, SMW)
                    nc.tensor.matmul(psh, ws2_t[:, cv, 2 * i:2 * i + 2, :], mv2,
                                     start=False, stop=False, perf_mode=DR)
                for j, t in enumerate(VSINGLES):
                    dy, dx = TAPS[t]
                    r0 = nch * 9 + dy
                    nc.tensor.matmul(psh, ws2_t[:, cv, 6 + j, :],
                                     sm_pad[:, 2, r0:r0 + 9, dx:dx + 32],
                                     start=False, stop=(j == 2))
                # zero the out-of-image halo row, then relu -> f8 (scaled S_H)
                nc.vector.tensor_tensor(
                    out=psh3, in0=psh3,
                    in1=hmask_bc[:, nch * 9:(nch + 1) * 9, None].to_broadcast([128, 9, 32]),
                    op=ALU.mult)
                nc.scalar.activation(
                    out=hp[:, nch * 9:(nch + 1) * 9, 1:33], in_=psh3,
                    func=AF.Relu, bias=bs_t[:, cv:cv + 1],
                    scale=S_H / (S_SM * S_WS))
            return hp

        # fold conv: out [64, 512] from hp windows, stationary wsm9 cols
        def fold_conv(hp, cols, pstile):
            for i, (ta, tb) in enumerate(VPAIRS):
                dy, dx = TAPS[ta]
                mv2 = _pair_ap(hp[:, dy:dy + 16, dx:dx + 32], SMW)
                nc.tensor.matmul(pstile, wsm9_t[:, 2 * i:2 * i + 2, cols], mv2,
                                 start=(i == 0), stop=False, perf_mode=DR)
            for j, t in enumerate(VSINGLES):
                dy, dx = TAPS[t]
                nc.tensor.matmul(pstile, wsm9_t[:, 6 + j, cols],
                                 hp[:, dy:dy + 16, dx:dx + 32],
                                 start=False, stop=(j == 2))

        h0p = h_conv(0)
        psC0 = PSC([64, 512], "psC0")
        fold_conv(h0p, slice(0, 64), psC0)
        psB0 = PSB([64, 512], "psB0")
        fold_conv(h0p, slice(64, 128), psB0)

        # ------------- LN0 partial stats (vector only; x ready by now) ------
        bno0 = dpool.tile([128, 12, 6], f32)
        for kc in range(12):
            nc.vector.bn_stats(out=bno0[:, kc, :], in_=xt[:, kc, :])
        mv0 = spool.tile([128, 2], f32, tag="mv0")
        nc.vector.bn_aggr(out=mv0, in_=bno0)
        mvE0 = spool.tile([128, 2], f32, tag="mvE0")
        nc.vector.tensor_tensor(out=mvE0[:, 0:1], in0=mv0[:, 0:1],
                                in1=mv0[:, 0:1], op=ALU.mult)
        nc.vector.tensor_tensor(out=mvE0[:, 1:2], in0=mv0[:, 1:2],
                                in1=mvE0[:, 0:1], op=ALU.add)
        nc.vector.tensor_scalar(out=mvE0[:, 0:1], in0=mv0[:, 0:1],
                                scalar1=1.0, scalar2=None, op0=ALU.mult)

        # partition-reduce (PE ones-matmul at an idle point) -> DRAM ->
        # AllReduce -> broadcast the raw sums back to all partitions
        def ln_stats(i, mvE, p_out, bc_tile):
            p_in = mvE.shape[0]
            pst = PSG([2, 1], f"pst{i}")
            nc.tensor.matmul(pst, mvE, ones_col[0:p_in, :],
                             start=True, stop=True)
            st_sb = spool.tile([2, 1], f32, tag=f"st_sb{i}")
            nc.scalar.copy(st_sb, pst)
            nc.gpsimd.dma_start(out=st_l[i][:], in_=st_sb[0:2, 0:1])
            nc.gpsimd.collective_compute(
                "AllReduce", ALU.add,
                replica_groups=[[0, 1], [2, 3], [4, 5], [6, 7]],
                ins=[st_l[i][:]], outs=[st_g[i][:]],
            )
            nc.gpsimd.dma_start(out=bc_tile,
                                in_=st_g[i][None, :].to_broadcast([p_out, 2]))

        sums0 = spool.tile([8, 2], f32, tag="sums0")

        # ---------------- conv_g + A0 pipeline ----------------
        gp1 = dpool.tile([128, 12, 512], f16)
        xg = dpool.tile([128, 12, 512], f16)
        psA0 = PSA([8, 512], "psA0")

        def conv_g_chunk(kc):
            psg = MAIN([128, 512], f"psg{kc}")
            for i, (ta, tb) in enumerate(VPAIRS):
                dy, dx = TAPS[ta]
                mv2 = _pair_ap(h0p[:, dy:dy + 16, dx:dx + 32], SMW)
                nc.tensor.matmul(psg, wg_t[:, kc, 2 * i:2 * i + 2, :], mv2,
                                 start=(i == 0), stop=False, perf_mode=DR)
            for j, t in enumerate(VSINGLES):
                dy, dx = TAPS[t]
                nc.tensor.matmul(psg, wg_t[:, kc, 6 + j, :],
                                 h0p[:, dy:dy + 16, dx:dx + 32],
                                 start=False, stop=(j == 2))
            nc.scalar.activation(out=gp1[:, kc, :], in_=psg, func=AF.Identity,
                                 bias=gb0_t[:, kc:kc + 1], scale=1.0 / (S_WG * S_H))
            nc.vector.tensor_tensor(out=xg[:, kc, :], in0=xt[:, kc, :],
                                    in1=gp1[:, kc, :], op=ALU.mult)

        def a0_mm(kc):
            nc.tensor.matmul(psA0, w0t_t[:, kc, :], xg[:, kc, :],
                             start=(kc == 0), stop=(kc == 11))

        for kc in range(12):
            conv_g_chunk(kc)
            if kc >= 2:
                a0_mm(kc - 2)
            if kc == 8:
                # LN0 stats round-trip launches while conv_g still runs
                ln_stats(0, mvE0, 8, sums0)
        a0_mm(10)
        a0_mm(11)

        # ---------------- SPADE1/2 convs (all stat-independent) -------------
        # psB1/psC2/psB2 folds are emitted later (PE tail): they reuse PSUM
        # banks whose previous tenants die only at the z0/z1 combines.
        h1p = h_conv(1)
        psg1 = PSG([64, 512], "psg1")
        fold_conv(h1p, slice(128, 192), psg1)
        gp11 = spool.tile([8, 512], f16, tag="gp11")
        nc.scalar.activation(out=gp11, in_=psg1[0:8, :], func=AF.Identity,
                             bias=gb1_t, scale=1.0 / (S_WG * S_H))
        psC1 = PSC([64, 512], "psC1")
        fold_conv(h1p, slice(192, 256), psC1)
        h2p = h_conv(2)
        psg2 = PSG([64, 512], "psg2")
        fold_conv(h2p, slice(320, 384), psg2)
        gp12 = spool.tile([16, 512], f16, tag="gp12")
        nc.scalar.activation(out=gp12, in_=psg2[0:16, :], func=AF.Identity,
                             bias=gb2_t, scale=1.0 / (S_WG * S_H))

        # ---------------- stats math + combine helpers ----------------
        def ln_finish(i, sums, p, n_inv, work_tag):
            """sums [p,2] broadcast raw sums -> work [p,4]: mu, r, -mu*r."""
            w = spool.tile([p, 6], f32, tag=work_tag)
            nc.vector.tensor_scalar_mul(w[:, 0:2], sums, n_inv)   # mu, E
            nc.vector.tensor_tensor(out=w[:, 2:3], in0=w[:, 0:1],
                                    in1=w[:, 0:1], op=ALU.mult)   # mu^2
            nc.vector.scalar_tensor_tensor(out=w[:, 3:4], in0=w[:, 2:3],
                                           scalar=-1.0, in1=w[:, 1:2],
                                           op0=ALU.mult, op1=ALU.add)  # var
            nc.vector.tensor_scalar(out=w[:, 3:4], in0=w[:, 3:4],
                                    scalar1=1.0, scalar2=1e-12,
                                    op0=ALU.mult, op1=ALU.add)
            nc.vector.reciprocal(out=w[:, 4:5], in_=w[:, 3:4])
            nc.scalar.sqrt(w[:, 5:6], w[:, 4:5])                  # r
            nc.vector.tensor_tensor(out=w[:, 2:3], in0=w[:, 0:1],
                                    in1=w[:, 5:6], op=ALU.mult)   # mu*r
            nc.vector.tensor_scalar_mul(w[:, 3:4], w[:, 2:3], -1.0)  # -mu*r
            return w  # r = w[:,5:6], negmur = w[:,3:4]

        def combine(psA, psC, psB, w, Bc, bf, sCB, p, tag):
            """z = r*A + sCB*((-mu r)*Bv + Cv) + (bf + (-mu r)*Bc)."""
            bias_dyn = spool.tile([p, 1], f32, tag=f"bd{tag}")
            nc.vector.scalar_tensor_tensor(out=bias_dyn, in0=Bc,
                                           scalar=w[:, 3:4], in1=bf,
                                           op0=ALU.mult, op1=ALU.add)
            tb = dpool.tile([p, 512], f32, tag=f"tb{tag}")
            nc.vector.tensor_scalar_mul(tb, psB, w[:, 3:4])
            t1 = dpool.tile([p, 512], f32, tag=f"t1{tag}")
            nc.vector.tensor_tensor(out=t1, in0=tb, in1=psC, op=ALU.add)
            t2 = dpool.tile([p, 512], f32, tag=f"t2{tag}")
            nc.vector.tensor_scalar(out=t2, in0=t1, scalar1=sCB,
                                    scalar2=bias_dyn, op0=ALU.mult, op1=ALU.add)
            z = dpool.tile([p, 512], f32, tag=f"z{tag}")
            nc.vector.scalar_tensor_tensor(out=z, in0=psA, scalar=w[:, 5:6],
                                           in1=t2, op0=ALU.mult, op1=ALU.add)
            return z

        # ---------------- layer 0 tail ----------------
        w0 = ln_finish(0, sums0, 8, 1.0 / 256.0, "w0")
        z0 = combine(psA0, psC0[0:8, :], psB0[0:8, :], w0, B0c_t, b0f_t,
                     1.0 / (S_F0 * S_H), 8, "0")
        out0 = dpool.tile([8, 512], f32)
        _softplus(nc, dpool, z0, out0, 8, "0")

        # og1 first (only needs out0), then LN1 stats, then PE tail folds
        og1 = spool.tile([8, 512], f16, tag="og1")
        nc.vector.tensor_tensor(out=og1, in0=out0, in1=gp11, op=ALU.mult)
        bno1 = spool.tile([8, 1, 6], f32, tag="bno1")
        nc.vector.bn_stats(out=bno1[:, 0, :], in_=out0)
        mv1 = spool.tile([8, 2], f32, tag="mv1")
        nc.vector.bn_aggr(out=mv1, in_=bno1)
        mvE1 = spool.tile([8, 2], f32, tag="mvE1")
        nc.vector.tensor_tensor(out=mvE1[:, 0:1], in0=mv1[:, 0:1],
                                in1=mv1[:, 0:1], op=ALU.mult)
        nc.vector.tensor_tensor(out=mvE1[:, 1:2], in0=mv1[:, 1:2],
                                in1=mvE1[:, 0:1], op=ALU.add)
        nc.vector.tensor_scalar(out=mvE1[:, 0:1], in0=mv1[:, 0:1],
                                scalar1=1.0, scalar2=None, op0=ALU.mult)

        psB1 = PSB([64, 512], "psB1")
        fold_conv(h1p, slice(256, 320), psB1)
        psC2 = PSC([64, 512], "psC2")
        fold_conv(h2p, slice(384, 448), psC2)
        psA1 = PSA([16, 512], "psA1")
        nc.tensor.matmul(psA1, w1t_t, og1, start=True, stop=True)
        sums1 = spool.tile([16, 2], f32, tag="sums1")
        ln_stats(1, mvE1, 16, sums1)

        # ---------------- layer 1 tail ----------------
        w1 = ln_finish(1, sums1, 16, 1.0 / 16.0, "w1")
        z1 = combine(psA1, psC1[0:16, :], psB1[0:16, :], w1, B1c_t, b1f_t,
                     1.0 / (S_F12 * S_H), 16, "1")
        out1 = dpool.tile([16, 512], f32)
        _softplus(nc, dpool, z1, out1, 16, "1")

        # og2 first, then LN2 stats; PE: psB2 fold, A2, pst2
        og2 = spool.tile([16, 512], f16, tag="og2")
        nc.vector.tensor_tensor(out=og2, in0=out1, in1=gp12, op=ALU.mult)
        bno2 = spool.tile([16, 1, 6], f32, tag="bno2")
        nc.vector.bn_stats(out=bno2[:, 0, :], in_=out1)
        mv2 = spool.tile([16, 2], f32, tag="mv2")
        nc.vector.bn_aggr(out=mv2, in_=bno2)
        mvE2 = spool.tile([16, 2], f32, tag="mvE2")
        nc.vector.tensor_tensor(out=mvE2[:, 0:1], in0=mv2[:, 0:1],
                                in1=mv2[:, 0:1], op=ALU.mult)
        nc.vector.tensor_tensor(out=mvE2[:, 1:2], in0=mv2[:, 1:2],
                                in1=mvE2[:, 0:1], op=ALU.add)
        nc.vector.tensor_scalar(out=mvE2[:, 0:1], in0=mv2[:, 0:1],
                                scalar1=1.0, scalar2=None, op0=ALU.mult)

        psB2 = PSB([64, 512], "psB2")
        fold_conv(h2p, slice(448, 512), psB2)
        psA2 = PSA([1, 512], "psA2")
        nc.tensor.matmul(psA2, w2t_t, og2, start=True, stop=True)
        sums2 = spool.tile([1, 2], f32, tag="sums2")
        ln_stats(2, mvE2, 1, sums2)

        # ---------------- layer 2 tail -> output ----------------
        w2 = ln_finish(2, sums2, 1, 1.0 / 32.0, "w2")
        z2 = combine(psA2, psC2[0:1, :], psB2[0:1, :], w2, B2c_t, b2f_t,
                     1.0 / (S_F12 * S_H), 1, "2")
        final = dpool.tile([1, 512], f32)
        _softplus(nc, dpool, z2, final, 1, "2")
        nc.scalar.dma_start(out=d_out[:], in_=final[0:1, :])

    nc.compile()
    return nc


def _host_prep(inputs):
    """Build per-core in_maps (host work: slicing, layout, small weight folds)."""
    x_main = np.asarray(inputs["x_main"], np.float32)
    f_sem = np.asarray(inputs["f_sem"], np.float32)
    seg = np.asarray(inputs["seg_mask"])
    f8np = ml_dtypes.float8_e4m3

    def lhsT9(w):  # [O, I, 3, 3] -> [I, 9, O]
        return np.ascontiguousarray(w.transpose(1, 2, 3, 0).reshape(w.shape[1], 9, w.shape[0]))

    ws_stack = np.stack([inputs["s0_ws"], inputs["s1_ws"], inputs["s2_ws"]])  # [3,128,384,3,3]
    ws_r = ws_stack.reshape(3, 128, 3, 128, 3, 3)          # cv, o, kc, i, ky, kx
    ws_full = (ws_r.transpose(3, 0, 2, 4, 5, 1)
               .reshape(128, 3, 3, 9, 128) * S_WS)         # [i, cv, kc, tap, o]
    WS01 = np.ascontiguousarray(
        ws_full[:, :, 0:2].transpose(0, 1, 3, 2, 4)).astype(f8np)  # [128,3,9,2,128]
    WS2 = np.ascontiguousarray(ws_full[:, :, 2][:, :, TAP_ORDER]).astype(f8np)
    wg0 = np.asarray(inputs["s0_wg"], np.float32)          # [1536, 128, 3, 3]
    WG = np.ascontiguousarray(
        (wg0.reshape(12, 128, 128, 3, 3).transpose(2, 0, 3, 4, 1)
         .reshape(128, 12, 9, 128) * S_WG)[:, :, TAP_ORDER]).astype(f8np)
    wf0 = np.einsum("oc,cikl->oikl", np.asarray(inputs["conv0_w"], np.float64),
                    np.asarray(inputs["s0_wb"], np.float64))
    wb0 = np.einsum("oc,cikl->oikl", np.asarray(inputs["conv0_w"], np.float64),
                    np.asarray(inputs["s0_wg"], np.float64))
    wf1 = np.einsum("oc,cikl->oikl", np.asarray(inputs["conv1_w"], np.float64),
                    np.asarray(inputs["s1_wb"], np.float64))
    wb1 = np.einsum("oc,cikl->oikl", np.asarray(inputs["conv1_w"], np.float64),
                    np.asarray(inputs["s1_wg"], np.float64))
    wf2 = np.einsum("oc,cikl->oikl", np.asarray(inputs["conv2_w"], np.float64),
                    np.asarray(inputs["s2_wb"], np.float64))
    wb2 = np.einsum("oc,cikl->oikl", np.asarray(inputs["conv2_w"], np.float64),
                    np.asarray(inputs["s2_wg"], np.float64))
    WSM9 = np.zeros((128, 9, 512), np.float64)
    WSM9[:, :, 0:8] = lhsT9(wf0) * S_F0
    WSM9[:, :, 64:72] = lhsT9(wb0) * S_F0
    WSM9[:, :, 128:136] = lhsT9(np.asarray(inputs["s1_wg"], np.float64)) * S_WG
    WSM9[:, :, 192:208] = lhsT9(wf1) * S_F12
    WSM9[:, :, 256:272] = lhsT9(wb1) * S_F12
    WSM9[:, :, 320:336] = lhsT9(np.asarray(inputs["s2_wg"], np.float64)) * S_WG
    WSM9[:, :, 384:385] = lhsT9(wf2) * S_F12
    WSM9[:, :, 448:449] = lhsT9(wb2) * S_F12
    WSM9 = WSM9[:, TAP_ORDER].astype(f8np)  # [128, 9, 512], tap-reordered
    W0T = np.ascontiguousarray(np.asarray(inputs["conv0_w"], np.float32).T
                               .reshape(12, 128, 8).transpose(1, 0, 2)).astype(np.float16)
    WSM = np.concatenate([
        np.asarray(inputs["conv1_w"], np.float32).T.reshape(-1),
        np.asarray(inputs["conv2_w"], np.float32).T.reshape(-1)]).astype(np.float16)  # [144]
    BSGB = np.concatenate([
        np.stack([inputs["s0_bs"], inputs["s1_bs"], inputs["s2_bs"]]).T * S_H,  # [128,3]
        (1.0 + np.asarray(inputs["s0_bg"], np.float32)).reshape(12, 128).T,     # [128,12]
    ], axis=1).astype(np.float32)                                               # [128,15]
    c0w = np.asarray(inputs["conv0_w"], np.float64)
    c1w = np.asarray(inputs["conv1_w"], np.float64)
    c2w = np.asarray(inputs["conv2_w"], np.float64)
    BIASV = np.zeros((16, 8), np.float32)
    BIASV[0:8, 0] = 1.0 + np.asarray(inputs["s1_bg"], np.float64)
    BIASV[0:16, 1] = 1.0 + np.asarray(inputs["s2_bg"], np.float64)
    BIASV[0:8, 2] = (np.asarray(inputs["b0"], np.float64)
                     + c0w @ np.asarray(inputs["s0_bb"], np.float64))
    BIASV[0:16, 3] = (np.asarray(inputs["b1"], np.float64)
                      + c1w @ np.asarray(inputs["s1_bb"], np.float64))
    BIASV[0:1, 4] = (np.asarray(inputs["b2"], np.float64)
                     + c2w @ np.asarray(inputs["s2_bb"], np.float64))
    BIASV[0:8, 5] = c0w @ (1.0 + np.asarray(inputs["s0_bg"], np.float64))
    BIASV[0:16, 6] = c1w @ (1.0 + np.asarray(inputs["s1_bg"], np.float64))
    BIASV[0:1, 7] = c2w @ (1.0 + np.asarray(inputs["s2_bg"], np.float64))

    shared = dict(ws01=WS01, ws2=WS2, wg=WG, wsm9=WSM9, w0t=W0T,
                  wsm=WSM, bsgb=BSGB, biasv=BIASV)

    in_maps = []
    for core in range(8):
        k, h = core // 2, core % 2
        r0 = HROWS * h
        X = np.ascontiguousarray(
            x_main[k, :, r0:r0 + HROWS, :].reshape(12, 128, 512).transpose(1, 0, 2)
        ).astype(np.float16)
        FT = np.ascontiguousarray(
            f_sem[k].reshape(384, NPOS).T.reshape(8, 128, 384).transpose(1, 0, 2)
        ).astype(np.float16)
        ids_flat = seg[k, ::14, ::14].astype(np.float32).reshape(NPOS)
        IDS = np.ascontiguousarray(ids_flat.reshape(8, 128).T)
        rows = np.arange(r0 - 2, r0 + HROWS + 2)          # 20 sm rows
        valid = (rows >= 0) & (rows < Hp)
        rcl = np.clip(rows, 0, Hp - 1)
        cid = np.empty((SMR, Wp, 4), np.float32)
        cols = np.arange(Wp)
        for t, (dy, dx) in enumerate([(0, 0), (0, 1), (1, 0), (1, 1)]):
            v = seg[k][np.ix_(14 * rcl + 6 + dy, 14 * cols + 6 + dx)].astype(np.float32)
            v[~valid, :] = -1.0
            cid[:, :, t] = v
        CID = np.ascontiguousarray(cid.reshape(5, 128, 4).transpose(1, 0, 2))
        hrows = np.arange(r0 - 1, r0 + HROWS + 1)
        HM = ((hrows >= 0) & (hrows < Hp)).astype(np.float32)
        in_maps.append(dict(shared, x=X, ft=FT, ids=IDS, cid=CID, hmask=HM))
    return in_maps


def kernel(**inputs):
    global _BUILT, LAST_RESULTS
    if _BUILT is None:
        _BUILT = _build_nc()
    nc = _BUILT
    in_maps = _host_prep(inputs)
    trace = bool(os.environ.get("BASS_TRACE"))
    res = run_bass_kernel_spmd(nc, in_maps, list(range(8)), trace=trace)
    LAST_RESULTS = res
    out = np.empty((B, 1, Hp, Wp), np.float32)
    for core in range(8):
        k, h = core // 2, core % 2
        out[k, 0, HROWS * h:HROWS * (h + 1), :] = \
            res.results[core]["out_half"].reshape(HROWS, Wp)
    return out


# revision 31
# speedup vs baseline: 1.1800x; 1.1441x over previous
"""Trainium2 Bass kernel for nn_DinoGazeSpade (segment_reduce + SPADE stack).

Layout: 8 cores; image k = core//2; each core computes rows [16h, 16h+16) of
the 32x32 grid (h = core%2). Cross-core: 3 pairwise AllReduces of LayerNorm
partial stats.

Structure (v2):
  - All 3x3 convs (sm->h, h->gp1, h->C/B folds, h->gp11/gp12) run as fp8e4m3
    DoubleRow matmuls (2 k-tiles per op) with host power-of-2 weight scaling;
    descale folded into the PSUM-read activation / combine scalars.
  - B terms (W @ gp) folded on host into conv weights from h (W@wg), merged
    into the same PSUM group as the C fold -> no gp-dependent projections.
  - LayerNorm stats never touch the PE: bn_stats/aggr (vector) ->
    gpsimd tensor_reduce(axis=C) -> DMA -> AllReduce -> broadcast-DMA of the
    raw sums to all partitions; rsqrt = vector.reciprocal + scalar Sqrt.
  - Scalar engine uses only {Sqrt, Relu, Identity, Copy, Square} => a single
    activation table set, no ACT_TABLE_LOAD ping-pong. softplus = relu(z) +
    rational(2,3)(min(|z|,8)) on the vector engine (max abs err ~7e-4).
  - PE program order: all stat-independent matmuls first (oh, sm, h0..h2,
    C/B folds, gp convs, conv_g+A0 pipelined), A1/A2 at the tail.
  - h-conv halo rows are zeroed by masking the PSUM (f32) before the relu.
"""
import os
import numpy as np
from contextlib import ExitStack

import ml_dtypes

import concourse.bass as bass
import concourse.mybir as mybir
import concourse.tile as tile
from concourse import bacc
from concourse.bass_utils import run_bass_kernel_spmd
from concourse.masks import make_identity

f32 = mybir.dt.float32
f16 = mybir.dt.float16
f8 = mybir.dt.float8e4
AF = mybir.ActivationFunctionType
ALU = mybir.AluOpType
AX = mybir.AxisListType
DR = mybir.MatmulPerfMode.DoubleRow

NSEG = 64
B, Cd, Hp, Wp, H, W, Cm, HID = 4, 384, 32, 32, 448, 448, 1536, 128
NPOS = Hp * Wp          # 1024
HROWS = 16              # rows per core
SMR = HROWS + 4         # sm rows incl 2-halo each side = 20
HR = HROWS + 2          # h rows incl 1-halo each side = 18
SMW = 48                # padded width (row stride must be 16B-aligned for dual-fp8)

# fp8 scale exponents (power of two)
S_SM = 8.0      # sm values
S_WS = 64.0     # ws weights
S_H = 4.0       # h values
S_WG = 64.0     # conv_g / wg1 / wg2 weights
S_F0 = 64.0     # layer-0 C/B fold weights
S_F12 = 256.0   # layer-1/2 C/B fold weights

# softplus tail g(t)=ln(1+exp(-t)) ~ (c0+c1 t+c2 t^2)/(1+d1 t+d2 t^2+d3 t^3)
# fit on t in [0,8] (t clamped at 8; g(8)=3.35e-4), max abs err 3.9e-4
SP_C0, SP_C1, SP_C2 = 0.6934867715618367, -0.17760652420286008, 0.011840728429853564
SP_D1, SP_D2, SP_D3 = 0.477190455932838, 0.1387482411055944, 0.0669674223997194

LAST_RESULTS = None  # set by kernel() for test harness introspection

_BUILT = None

TAPS = [(t // 3, t % 3) for t in range(9)]
# Dual-fp8 moving APs need all outer steps 16B-aligned: pair taps VERTICALLY
# (delta = row stride 48). Pairs (t, t+3) for t in 0..2; taps 6..8 single.
# Weight tensors store taps in order [0,3,1,4,2,5,6,7,8] so pairs and the
# k-tile dim are adjacent.
VPAIRS = [(0, 3), (1, 4), (2, 5)]
VSINGLES = [6, 7, 8]
TAP_ORDER = [0, 3, 1, 4, 2, 5, 6, 7, 8]


def _pair_ap(a, delta):
    """Insert a [delta, 2] k-tile dim as dim 1 of an AP (for DoubleRow rhs)."""
    ap = list(a.ap)
    new = [ap[0], [delta, 2]] + list(ap[1:])
    return bass.AP(a.tensor, a.offset, new)


def _softplus(nc, pool, z, out_tile, p, tag):
    """out = softplus(z) = relu(z) + g(min(|z|,8)); scalar does the relu."""
    ta = pool.tile([p, 512], f32, tag=f"sp_ta{tag}")
    nc.scalar.activation(out=ta, in_=z, func=AF.Abs, bias=0.0)
    t = pool.tile([p, 512], f32, tag=f"sp_t{tag}")
    nc.vector.tensor_scalar(out=t, in0=ta, scalar1=8.0, scalar2=None,
                            op0=ALU.min)
    t2 = pool.tile([p, 512], f32, tag=f"sp_t2{tag}")
    nc.vector.tensor_tensor(out=t2, in0=t, in1=t, op=ALU.mult)
    rl = pool.tile([p, 512], f32, tag=f"sp_rl{tag}")
    nc.scalar.activation(out=rl, in_=z, func=AF.Relu, bias=0.0)
    n1 = pool.tile([p, 512], f32, tag=f"sp_n1{tag}")
    nc.vector.tensor_scalar(out=n1, in0=t, scalar1=SP_C1, scalar2=SP_C0,
                            op0=ALU.mult, op1=ALU.add)
    num = pool.tile([p, 512], f32, tag=f"sp_num{tag}")
    nc.vector.scalar_tensor_tensor(out=num, in0=t2, scalar=SP_C2, in1=n1,
                                   op0=ALU.mult, op1=ALU.add)
    d1p = pool.tile([p, 512], f32, tag=f"sp_d1{tag}")
    nc.vector.tensor_scalar(out=d1p, in0=t, scalar1=SP_D1, scalar2=1.0,
                            op0=ALU.mult, op1=ALU.add)
    q = pool.tile([p, 512], f32, tag=f"sp_q{tag}")
    nc.vector.tensor_scalar(out=q, in0=t, scalar1=SP_D3, scalar2=SP_D2,
                            op0=ALU.mult, op1=ALU.add)
    den = pool.tile([p, 512], f32, tag=f"sp_den{tag}")
    nc.vector.tensor_tensor(out=den, in0=q, in1=t2, op=ALU.mult)
    nc.vector.tensor_tensor(out=den, in0=den, in1=d1p, op=ALU.add)
    rd = pool.tile([p, 512], f32, tag=f"sp_rd{tag}")
    nc.vector.reciprocal_approx_fast(out=rd, in_=den)
    gg = pool.tile([p, 512], f32, tag=f"sp_gg{tag}")
    nc.vector.tensor_tensor(out=gg, in0=num, in1=rd, op=ALU.mult)
    nc.vector.tensor_tensor(out=out_tile, in0=gg, in1=rl, op=ALU.add)


def _build_nc():
    stage = int(os.environ.get("KBISECT", "99"))
    nc = bacc.Bacc("TRN2", num_devices=8)

    # (f32, 0.0) const AP is pre-registered by the framework
    nc.all_engine_barrier()

    # ---------------- DRAM I/O ----------------
    d_x = nc.dram_tensor("x", [128, 12, 512], f16, kind="ExternalInput")
    d_ft = nc.dram_tensor("ft", [128, 8, 384], f16, kind="ExternalInput")
    d_ids = nc.dram_tensor("ids", [128, 8], f32, kind="ExternalInput")
    d_cid = nc.dram_tensor("cid", [128, 5, 4], f32, kind="ExternalInput")
    d_hmask = nc.dram_tensor("hmask", [HR], f32, kind="ExternalInput")
    d_ws01 = nc.dram_tensor("ws01", [128, 3, 9, 2, 128], f8, kind="ExternalInput")
    d_ws2 = nc.dram_tensor("ws2", [128, 3, 9, 128], f8, kind="ExternalInput")
    d_wg = nc.dram_tensor("wg", [128, 12, 9, 128], f8, kind="ExternalInput")
    # wsm9: 8 groups of 64 cols (dual-fp8 w/ windowed moving wants >=64):
    # wf0@0(8) wb0@64(8) wg1@128(8) wf1@192(16) wb1@256(16) wg2@320(16)
    # wf2@384(1) wb2@448(1)
    d_wsm9 = nc.dram_tensor("wsm9", [128, 9, 512], f8, kind="ExternalInput")
    d_w0t = nc.dram_tensor("w0t", [128, 12, 8], f16, kind="ExternalInput")
    d_wsm = nc.dram_tensor("wsm", [144], f16, kind="ExternalInput")  # w1t|w2t
    d_bsgb = nc.dram_tensor("bsgb", [128, 15], f32, kind="ExternalInput")
    # biasv [16,8] cols: gb1 gb2 b0f b1f b2f B0c B1c B2c (each from row 0)
    d_biasv = nc.dram_tensor("biasv", [16, 8], f32, kind="ExternalInput")
    d_out = nc.dram_tensor("out_half", [512], f32, kind="ExternalOutput")

    st_l = [nc.dram_tensor(f"st{i}_l", [2], f32) for i in range(3)]
    st_g = [nc.dram_tensor(f"st{i}_g", [2], f32) for i in range(3)]

    with ExitStack() as ctx:
        tc = ctx.enter_context(tile.TileContext(nc, num_cores=8))
        cpool = ctx.enter_context(tc.tile_pool(name="consts", bufs=1))
        dpool = ctx.enter_context(tc.tile_pool(name="data", bufs=1))
        spool = ctx.enter_context(tc.tile_pool(name="small", bufs=1))
        ps = ctx.enter_context(tc.tile_pool(name="ps", bufs=1, space="PSUM"))

        def MAIN(shape, name):
            return ps.tile(shape, f32, tag="ps_main", bufs=2, name=name)

        def PSA(shape, name):  # psA0 -> psA1 -> psA2
            return ps.tile(shape, f32, tag="ps_a", bufs=1, name=name)

        def PSC(shape, name):  # psC0 -> psC1 -> psC2
            return ps.tile(shape, f32, tag="ps_c", bufs=2, name=name)

        def PSB(shape, name):  # seg sums -> psB0 -> psB1 -> psB2 (bank reuse)
            return ps.tile(shape, f32, tag="ps_sums", bufs=1, name=name)

        def PSG(shape, name, dt=f32):  # gr transposes, psg1, psg2
            return ps.tile(shape, dt, tag="ps_g", bufs=2, name=name)

        # ---- gpsimd constants first (iota gates the OH build) ----
        iot = cpool.tile([128, 64], f32)
        nc.gpsimd.iota(iot, pattern=[[1, 64]], base=0, channel_multiplier=0,
                       allow_small_or_imprecise_dtypes=True)
        ident = cpool.tile([128, 128], f16)
        make_identity(nc, ident)
        ones_col = cpool.tile([128, 1], f32)
        nc.gpsimd.memset(ones_col, 1.0)

        # --------- DMAs: sync queue = big early tensors, in need order ------
        idst = cpool.tile([128, 8], f32)
        nc.sync.dma_start(out=idst, in_=d_ids[:, :])
        cidt = cpool.tile([128, 5, 4], f32)
        nc.sync.dma_start(out=cidt, in_=d_cid[:, :, :])
        feats = dpool.tile([128, 8, 385], f16)
        nc.sync.dma_start(out=feats[:, 0:4, 0:384], in_=d_ft[:, 0:4, :])
        nc.sync.dma_start(out=feats[:, 4:8, 0:384], in_=d_ft[:, 4:8, :])
        xt = dpool.tile([128, 12, 512], f16)
        nc.sync.dma_start(out=xt, in_=d_x[:, :, :])
        ws01_t = cpool.tile([128, 3, 9, 2, 128], f8)
        nc.sync.dma_start(out=ws01_t[:, 0:1], in_=d_ws01[:, 0:1])  # s0_ws first
        ws2_t = cpool.tile([128, 3, 9, 128], f8)
        nc.sync.dma_start(out=ws2_t[:, 0:1], in_=d_ws2[:, 0:1])
        wg_t = cpool.tile([128, 12, 9, 128], f8)
        for g in range(3):
            nc.sync.dma_start(out=wg_t[:, g * 4:(g + 1) * 4],
                              in_=d_wg[:, g * 4:(g + 1) * 4])
        nc.sync.dma_start(out=ws01_t[:, 1:3], in_=d_ws01[:, 1:3])  # s1/s2_ws
        nc.sync.dma_start(out=ws2_t[:, 1:3], in_=d_ws2[:, 1:3])

        # --------- small/later tensors issued from the scalar queue ---------
        wsm9_t = cpool.tile([128, 9, 512], f8)
        nc.scalar.dma_start(out=wsm9_t, in_=d_wsm9[:, :, :])
        w0t_t = cpool.tile([128, 12, 8], f16)
        nc.scalar.dma_start(out=w0t_t, in_=d_w0t[:, :, :])
        bsgb_t = cpool.tile([128, 15], f32)
        nc.scalar.dma_start(out=bsgb_t, in_=d_bsgb[:, :])
        bs_t = bsgb_t[:, 0:3]          # S_H * bs, per conv
        gb0_t = bsgb_t[:, 3:15]        # 1 + bg0
        w1t_t = cpool.tile([8, 16], f16)
        nc.scalar.dma_start(out=w1t_t, in_=d_wsm[0:128].rearrange("(a b) -> a b", b=16))
        w2t_t = cpool.tile([16, 1], f16)
        nc.scalar.dma_start(out=w2t_t, in_=d_wsm[128:144][:, None])
        biasv_t = cpool.tile([16, 8], f32)
        nc.scalar.dma_start(out=biasv_t, in_=d_biasv[:, :])
        gb1_t = biasv_t[0:8, 0:1]
        gb2_t = biasv_t[0:16, 1:2]
        b0f_t = biasv_t[0:8, 2:3]
        b1f_t = biasv_t[0:16, 3:4]
        b2f_t = biasv_t[0:1, 4:5]
        B0c_t = biasv_t[0:8, 5:6]
        B1c_t = biasv_t[0:16, 6:7]
        B2c_t = biasv_t[0:1, 7:8]
        hmask_bc = cpool.tile([128, HR], f32)
        nc.scalar.dma_start(out=hmask_bc, in_=d_hmask[None, :].to_broadcast([128, HR]))

        # ---- engine warmups during the DMA window ----
        warm = cpool.tile([128, 64], f32)
        for _ in range(3):
            nc.vector.memset(warm, 0.0)
        pswarm = MAIN([128, 128], "pswarm")
        for _ in range(6):
            nc.tensor.matmul(pswarm, ident, ident, start=True, stop=True)
        wread = cpool.tile([128, 128], f32)
        nc.scalar.copy(wread, pswarm)

        nc.gpsimd.memset(feats[:, :, 384:385], 1.0)

        # ---------------- segment one-hots (vector) ----------------
        oh_t = dpool.tile([128, 8, 64], f16)
        for qc in range(8):
            nc.vector.tensor_scalar(out=oh_t[:, qc, :], in0=iot,
                                    scalar1=idst[:, qc:qc + 1], scalar2=None,
                                    op0=ALU.is_equal)
        gacc = dpool.tile([128, 5, 64], f16)
        for jc in range(5):
            nc.vector.tensor_scalar(out=gacc[:, jc, :], in0=iot,
                                    scalar1=cidt[:, jc, 0:1], scalar2=None,
                                    op0=ALU.is_equal)
            for corner in range(1, 4):
                nc.vector.scalar_tensor_tensor(
                    out=gacc[:, jc, :], in0=iot,
                    scalar=cidt[:, jc, corner:corner + 1],
                    in1=gacc[:, jc, :], op0=ALU.is_equal, op1=ALU.add)

        # ---------------- segment means avg' [64, 384] ----------------
        psums = ps.tile([64, 385], f32, tag="ps_sums", bufs=1)
        for qc in range(8):
            nc.tensor.matmul(psums, oh_t[:, qc, :], feats[:, qc, :],
                             start=(qc == 0), stop=(qc == 7))
        cnt4 = spool.tile([64, 1], f32, tag="cnt4")
        nc.vector.tensor_scalar(out=cnt4, in0=psums[:, 384:385], scalar1=1.0,
                                scalar2=4.0, op0=ALU.max, op1=ALU.mult)
        recip4 = spool.tile([64, 1], f32, tag="recip4")
        nc.vector.reciprocal(out=recip4, in_=cnt4)
        avg_t = dpool.tile([64, 384], f16)
        nc.vector.tensor_scalar_mul(avg_t, psums[:, 0:384], recip4[:, 0:1])

        # ---------------- G masks -> Gr [64, 640] ----------------
        gr_t = dpool.tile([64, 640], f16)
        for jc in range(5):
            ptr = PSG([64, 128], f"ptr{jc}", dt=f16)
            nc.tensor.transpose(ptr, gacc[:, jc, :], ident)
            nc.scalar.copy(gr_t[:, jc * 128:(jc + 1) * 128], ptr)

        # ---------------- sm (f8, scaled by S_SM) ----------------
        sm_pad = dpool.tile([128, 3, SMR, SMW], f8)
        nc.gpsimd.memset(sm_pad, 0.0)
        for mc in range(3):
            for nch in range(2):
                psm = MAIN([128, 320], f"psm{mc}{nch}")
                nc.tensor.matmul(psm, avg_t[:, mc * 128:(mc + 1) * 128],
                                 gr_t[:, nch * 320:(nch + 1) * 320],
                                 start=True, stop=True)
                nc.scalar.activation(
                    out=sm_pad[:, mc, nch * 10:(nch + 1) * 10, 1:33],
                    in_=psm.rearrange("p (r c) -> p r c", c=32),
                    func=AF.Copy, scale=S_SM)

        # ---------------- h convs (fp8 DoubleRow over kc pairs + taps) ------
        hps = []

        def h_conv(cv):
            hp = dpool.tile([128, HR, SMW], f8, tag=f"hpad{cv}", name=f"hpad{cv}")
            nc.gpsimd.memset(hp, 0.0)
            for nch in range(2):
                psh = MAIN([128, 9 * 32], f"psh{cv}{nch}")
                psh3 = psh.rearrange("p (r c) -> p r c", c=32)
                # kc-pair (0,1) DoubleRow per tap (k-tile delta = plane stride)
                for t, (dy, dx) in enumerate(TAPS):
                    r0 = nch * 9 + dy
                    mv = sm_pad[:, 0, r0:r0 + 9, dx:dx + 32]
                    mv2 = _pair_ap(mv, SMR * SMW)
                    nc.tensor.matmul(psh, ws01_t[:, cv, t, :, :], mv2,
                                     start=(t == 0), stop=False, perf_mode=DR)
                # kc=2: vertical tap-pairs (delta = row stride) + 3 singles
                for i, (ta, tb) in enumerate(VPAIRS):
                    dy, dx = TAPS[ta]
                    r0 = nch * 9 + dy
                    mv = sm_pad[:, 2, r0:r0 + 9, dx:dx + 32]
                    mv2 = _pair_ap(mv, SMW)
                    nc.tensor.matmul(psh, ws2_t[:, cv, 2 * i:2 * i + 2, :], mv2,
                                     start=False, stop=False, perf_mode=DR)
                for j, t in enumerate(VSINGLES):
                    dy, dx = TAPS[t]
                    r0 = nch * 9 + dy
                    nc.tensor.matmul(psh, ws2_t[:, cv, 6 + j, :],
                                     sm_pad[:, 2, r0:r0 + 9, dx:dx + 32],
                                     start=False, stop=(j == 2))
                # zero the out-of-image halo row, then relu -> f8 (scaled S_H)
                nc.vector.tensor_tensor(
                    out=psh3, in0=psh3,
                    in1=hmask_bc[:, nch * 9:(nch + 1) * 9, None].to_broadcast([128, 9, 32]),
                    op=ALU.mult)
                nc.scalar.activation(
                    out=hp[:, nch * 9:(nch + 1) * 9, 1:33], in_=psh3,
                    func=AF.Relu, bias=bs_t[:, cv:cv + 1],
                    scale=S_H / (S_SM * S_WS))
            return hp

        # fold conv: out [64, 512] from hp windows, stationary wsm9 cols
        def fold_conv(hp, cols, pstile):
            for i, (ta, tb) in enumerate(VPAIRS):
                dy, dx = TAPS[ta]
                mv2 = _pair_ap(hp[:, dy:dy + 16, dx:dx + 32], SMW)
                nc.tensor.matmul(pstile, wsm9_t[:, 2 * i:2 * i + 2, cols], mv2,
                                 start=(i == 0), stop=False, perf_mode=DR)
            for j, t in enumerate(VSINGLES):
                dy, dx = TAPS[t]
                nc.tensor.matmul(pstile, wsm9_t[:, 6 + j, cols],
                                 hp[:, dy:dy + 16, dx:dx + 32],
                                 start=False, stop=(j == 2))

        # ------------- LN0 partial stats (vector; x lands early now) -------
        bno0 = dpool.tile([128, 12, 6], f32)
        for kc in range(12):
            nc.vector.bn_stats(out=bno0[:, kc, :], in_=xt[:, kc, :])
        mv0 = spool.tile([128, 2], f32, tag="mv0")
        nc.vector.bn_aggr(out=mv0, in_=bno0)
        mvE0 = spool.tile([128, 2], f32, tag="mvE0")
        nc.vector.tensor_tensor(out=mvE0[:, 0:1], in0=mv0[:, 0:1],
                                in1=mv0[:, 0:1], op=ALU.mult)
        nc.vector.tensor_tensor(out=mvE0[:, 1:2], in0=mv0[:, 1:2],
                                in1=mvE0[:, 0:1], op=ALU.add)
        nc.vector.tensor_scalar(out=mvE0[:, 0:1], in0=mv0[:, 0:1],
                                scalar1=1.0, scalar2=None, op0=ALU.mult)

        h0p = h_conv(0)
        psC0 = PSC([64, 512], "psC0")
        fold_conv(h0p, slice(0, 64), psC0)
        psB0 = PSB([64, 512], "psB0")
        fold_conv(h0p, slice(64, 128), psB0)

        # partition-reduce (PE ones-matmul at an idle point) -> DRAM ->
        # AllReduce -> broadcast the raw sums back to all partitions
        def ln_stats(i, mvE, p_out, bc_tile):
            p_in = mvE.shape[0]
            pst = PSG([2, 1], f"pst{i}")
            nc.tensor.matmul(pst, mvE, ones_col[0:p_in, :],
                             start=True, stop=True)
            st_sb = spool.tile([2, 1], f32, tag=f"st_sb{i}")
            nc.vector.tensor_copy(out=st_sb, in_=pst)
            nc.gpsimd.dma_start(out=st_l[i][:], in_=st_sb[0:2, 0:1])
            nc.gpsimd.collective_compute(
                "AllReduce", ALU.add,
                replica_groups=[[0, 1], [2, 3], [4, 5], [6, 7]],
                ins=[st_l[i][:]], outs=[st_g[i][:]],
            )
            nc.gpsimd.dma_start(out=bc_tile,
                                in_=st_g[i][None, :].to_broadcast([p_out, 2]))

        sums0 = spool.tile([8, 2], f32, tag="sums0")

        # ---------------- conv_g + A0 pipeline ----------------
        gp1 = dpool.tile([128, 12, 512], f16)
        xg = dpool.tile([128, 12, 512], f16)
        psA0 = PSA([8, 512], "psA0")

        def conv_g_chunk(kc):
            psg = MAIN([128, 512], f"psg{kc}")
            for i, (ta, tb) in enumerate(VPAIRS):
                dy, dx = TAPS[ta]
                mv2 = _pair_ap(h0p[:, dy:dy + 16, dx:dx + 32], SMW)
                nc.tensor.matmul(psg, wg_t[:, kc, 2 * i:2 * i + 2, :], mv2,
                                 start=(i == 0), stop=False, perf_mode=DR)
            for j, t in enumerate(VSINGLES):
                dy, dx = TAPS[t]
                nc.tensor.matmul(psg, wg_t[:, kc, 6 + j, :],
                                 h0p[:, dy:dy + 16, dx:dx + 32],
                                 start=False, stop=(j == 2))
            nc.scalar.activation(out=gp1[:, kc, :], in_=psg, func=AF.Identity,
                                 bias=gb0_t[:, kc:kc + 1], scale=1.0 / (S_WG * S_H))
            nc.gpsimd.tensor_tensor(out=xg[:, kc, :], in0=xt[:, kc, :],
                                     in1=gp1[:, kc, :], op=ALU.mult)

        def a0_mm(kc):
            nc.tensor.matmul(psA0, w0t_t[:, kc, :], xg[:, kc, :],
                             start=(kc == 0), stop=(kc == 11))

        for kc in range(12):
            conv_g_chunk(kc)
            if kc >= 2:
                a0_mm(kc - 2)
            if kc == 2:
                # LN0 stats round-trip launches while conv_g still runs
                ln_stats(0, mvE0, 8, sums0)
        a0_mm(10)
        a0_mm(11)

        # ---------------- SPADE1/2 convs (all stat-independent) -------------
        # psB1/psC2/psB2 folds are emitted later (PE tail): they reuse PSUM
        # banks whose previous tenants die only at the z0/z1 combines.
        h1p = h_conv(1)
        psg1 = PSG([64, 512], "psg1")
        fold_conv(h1p, slice(128, 192), psg1)
        gp11 = spool.tile([8, 512], f16, tag="gp11")
        nc.scalar.activation(out=gp11, in_=psg1[0:8, :], func=AF.Identity,
                             bias=gb1_t, scale=1.0 / (S_WG * S_H))
        psC1 = PSC([64, 512], "psC1")
        fold_conv(h1p, slice(192, 256), psC1)
        h2p = h_conv(2)
        psg2 = PSG([64, 512], "psg2")
        fold_conv(h2p, slice(320, 384), psg2)
        gp12 = spool.tile([16, 512], f16, tag="gp12")
        nc.scalar.activation(out=gp12, in_=psg2[0:16, :], func=AF.Identity,
                             bias=gb2_t, scale=1.0 / (S_WG * S_H))

        # ---------------- stats math + combine helpers ----------------
        def ln_finish(i, sums, p, n_inv, work_tag):
            """sums [p,2] broadcast raw sums -> work [p,4]: mu, r, -mu*r."""
            w = spool.tile([p, 6], f32, tag=work_tag)
            nc.vector.tensor_scalar_mul(w[:, 0:2], sums, n_inv)   # mu, E
            nc.vector.tensor_tensor(out=w[:, 2:3], in0=w[:, 0:1],
                                    in1=w[:, 0:1], op=ALU.mult)   # mu^2
            nc.vector.scalar_tensor_tensor(out=w[:, 3:4], in0=w[:, 2:3],
                                           scalar=-1.0, in1=w[:, 1:2],
                                           op0=ALU.mult, op1=ALU.add)  # var
            nc.vector.tensor_scalar(out=w[:, 3:4], in0=w[:, 3:4],
                                    scalar1=1.0, scalar2=1e-12,
                                    op0=ALU.mult, op1=ALU.add)
            nc.vector.reciprocal_approx_fast(out=w[:, 4:5], in_=w[:, 3:4])
            nc.scalar.sqrt(w[:, 5:6], w[:, 4:5])                  # r
            nc.vector.tensor_tensor(out=w[:, 2:3], in0=w[:, 0:1],
                                    in1=w[:, 5:6], op=ALU.mult)   # mu*r
            nc.vector.tensor_scalar_mul(w[:, 3:4], w[:, 2:3], -1.0)  # -mu*r
            return w  # r = w[:,5:6], negmur = w[:,3:4]

        def combine(psA, psC, psB, w, Bc, bf, sCB, p, tag):
            """z = r*A + sCB*((-mu r)*Bv + Cv) + (bf + (-mu r)*Bc)."""
            bias_dyn = spool.tile([p, 1], f32, tag=f"bd{tag}")
            nc.vector.scalar_tensor_tensor(out=bias_dyn, in0=Bc,
                                           scalar=w[:, 3:4], in1=bf,
                                           op0=ALU.mult, op1=ALU.add)
            tb = dpool.tile([p, 512], f32, tag=f"tb{tag}")
            nc.vector.tensor_scalar_mul(tb, psB, w[:, 3:4])
            t1 = dpool.tile([p, 512], f32, tag=f"t1{tag}")
            nc.vector.tensor_tensor(out=t1, in0=tb, in1=psC, op=ALU.add)
            t2 = dpool.tile([p, 512], f32, tag=f"t2{tag}")
            nc.vector.tensor_scalar(out=t2, in0=t1, scalar1=sCB,
                                    scalar2=bias_dyn, op0=ALU.mult, op1=ALU.add)
            z = dpool.tile([p, 512], f32, tag=f"z{tag}")
            nc.vector.scalar_tensor_tensor(out=z, in0=psA, scalar=w[:, 5:6],
                                           in1=t2, op0=ALU.mult, op1=ALU.add)
            return z

        # ---------------- layer 0 tail ----------------
        w0 = ln_finish(0, sums0, 8, 1.0 / 256.0, "w0")
        z0 = combine(psA0, psC0[0:8, :], psB0[0:8, :], w0, B0c_t, b0f_t,
                     1.0 / (S_F0 * S_H), 8, "0")
        out0 = dpool.tile([8, 512], f32)
        _softplus(nc, dpool, z0, out0, 8, "0")

        # og1 first (only needs out0), then LN1 stats, then PE tail folds
        og1 = spool.tile([8, 512], f16, tag="og1")
        nc.vector.tensor_tensor(out=og1, in0=out0, in1=gp11, op=ALU.mult)
        bno1 = spool.tile([8, 1, 6], f32, tag="bno1")
        nc.vector.bn_stats(out=bno1[:, 0, :], in_=out0)
        mv1 = spool.tile([8, 2], f32, tag="mv1")
        nc.vector.bn_aggr(out=mv1, in_=bno1)
        mvE1 = spool.tile([8, 2], f32, tag="mvE1")
        nc.vector.tensor_tensor(out=mvE1[:, 0:1], in0=mv1[:, 0:1],
                                in1=mv1[:, 0:1], op=ALU.mult)
        nc.vector.tensor_tensor(out=mvE1[:, 1:2], in0=mv1[:, 1:2],
                                in1=mvE1[:, 0:1], op=ALU.add)
        nc.vector.tensor_scalar(out=mvE1[:, 0:1], in0=mv1[:, 0:1],
                                scalar1=1.0, scalar2=None, op0=ALU.mult)

        psB1 = PSB([64, 512], "psB1")
        fold_conv(h1p, slice(256, 320), psB1)
        psC2 = PSC([64, 512], "psC2")
        fold_conv(h2p, slice(384, 448), psC2)
        psA1 = PSA([16, 512], "psA1")
        nc.tensor.matmul(psA1, w1t_t, og1, start=True, stop=True)
        sums1 = spool.tile([16, 2], f32, tag="sums1")
        ln_stats(1, mvE1, 16, sums1)

        # ---------------- layer 1 tail ----------------
        w1 = ln_finish(1, sums1, 16, 1.0 / 16.0, "w1")
        z1 = combine(psA1, psC1[0:16, :], psB1[0:16, :], w1, B1c_t, b1f_t,
                     1.0 / (S_F12 * S_H), 16, "1")
        out1 = dpool.tile([16, 512], f32)
        _softplus(nc, dpool, z1, out1, 16, "1")

        # og2 first, then LN2 stats; PE: psB2 fold, A2, pst2
        og2 = spool.tile([16, 512], f16, tag="og2")
        nc.vector.tensor_tensor(out=og2, in0=out1, in1=gp12, op=ALU.mult)
        bno2 = spool.tile([16, 1, 6], f32, tag="bno2")
        nc.vector.bn_stats(out=bno2[:, 0, :], in_=out1)
        mv2 = spool.tile([16, 2], f32, tag="mv2")
        nc.vector.bn_aggr(out=mv2, in_=bno2)
        mvE2 = spool.tile([16, 2], f32, tag="mvE2")
        nc.vector.tensor_tensor(out=mvE2[:, 0:1], in0=mv2[:, 0:1],
                                in1=mv2[:, 0:1], op=ALU.mult)
        nc.vector.tensor_tensor(out=mvE2[:, 1:2], in0=mv2[:, 1:2],
                                in1=mvE2[:, 0:1], op=ALU.add)
        nc.vector.tensor_scalar(out=mvE2[:, 0:1], in0=mv2[:, 0:1],
                                scalar1=1.0, scalar2=None, op0=ALU.mult)

        psB2 = PSB([64, 512], "psB2")
        fold_conv(h2p, slice(448, 512), psB2)
        psA2 = PSA([1, 512], "psA2")
        nc.tensor.matmul(psA2, w2t_t, og2, start=True, stop=True)
        sums2 = spool.tile([1, 2], f32, tag="sums2")
        ln_stats(2, mvE2, 1, sums2)

        # ---------------- layer 2 tail -> output ----------------
        w2 = ln_finish(2, sums2, 1, 1.0 / 32.0, "w2")
        z2 = combine(psA2, psC2[0:1, :], psB2[0:1, :], w2, B2c_t, b2f_t,
                     1.0 / (S_F12 * S_H), 1, "2")
        final = dpool.tile([1, 512], f32)
        _softplus(nc, dpool, z2, final, 1, "2")
        nc.scalar.dma_start(out=d_out[:], in_=final[0:1, :])

    nc.compile()
    return nc


def _host_prep(inputs):
    """Build per-core in_maps (host work: slicing, layout, small weight folds)."""
    x_main = np.asarray(inputs["x_main"], np.float32)
    f_sem = np.asarray(inputs["f_sem"], np.float32)
    seg = np.asarray(inputs["seg_mask"])
    f8np = ml_dtypes.float8_e4m3

    def lhsT9(w):  # [O, I, 3, 3] -> [I, 9, O]
        return np.ascontiguousarray(w.transpose(1, 2, 3, 0).reshape(w.shape[1], 9, w.shape[0]))

    ws_stack = np.stack([inputs["s0_ws"], inputs["s1_ws"], inputs["s2_ws"]])  # [3,128,384,3,3]
    ws_r = ws_stack.reshape(3, 128, 3, 128, 3, 3)          # cv, o, kc, i, ky, kx
    ws_full = (ws_r.transpose(3, 0, 2, 4, 5, 1)
               .reshape(128, 3, 3, 9, 128) * S_WS)         # [i, cv, kc, tap, o]
    WS01 = np.ascontiguousarray(
        ws_full[:, :, 0:2].transpose(0, 1, 3, 2, 4)).astype(f8np)  # [128,3,9,2,128]
    WS2 = np.ascontiguousarray(ws_full[:, :, 2][:, :, TAP_ORDER]).astype(f8np)
    wg0 = np.asarray(inputs["s0_wg"], np.float32)          # [1536, 128, 3, 3]
    WG = np.ascontiguousarray(
        (wg0.reshape(12, 128, 128, 3, 3).transpose(2, 0, 3, 4, 1)
         .reshape(128, 12, 9, 128) * S_WG)[:, :, TAP_ORDER]).astype(f8np)
    wf0 = np.einsum("oc,cikl->oikl", np.asarray(inputs["conv0_w"], np.float64),
                    np.asarray(inputs["s0_wb"], np.float64))
    wb0 = np.einsum("oc,cikl->oikl", np.asarray(inputs["conv0_w"], np.float64),
                    np.asarray(inputs["s0_wg"], np.float64))
    wf1 = np.einsum("oc,cikl->oikl", np.asarray(inputs["conv1_w"], np.float64),
                    np.asarray(inputs["s1_wb"], np.float64))
    wb1 = np.einsum("oc,cikl->oikl", np.asarray(inputs["conv1_w"], np.float64),
                    np.asarray(inputs["s1_wg"], np.float64))
    wf2 = np.einsum("oc,cikl->oikl", np.asarray(inputs["conv2_w"], np.float64),
                    np.asarray(inputs["s2_wb"], np.float64))
    wb2 = np.einsum("oc,cikl->oikl", np.asarray(inputs["conv2_w"], np.float64),
                    np.asarray(inputs["s2_wg"], np.float64))
    WSM9 = np.zeros((128, 9, 512), np.float64)
    WSM9[:, :, 0:8] = lhsT9(wf0) * S_F0
    WSM9[:, :, 64:72] = lhsT9(wb0) * S_F0
    WSM9[:, :, 128:136] = lhsT9(np.asarray(inputs["s1_wg"], np.float64)) * S_WG
    WSM9[:, :, 192:208] = lhsT9(wf1) * S_F12
    WSM9[:, :, 256:272] = lhsT9(wb1) * S_F12
    WSM9[:, :, 320:336] = lhsT9(np.asarray(inputs["s2_wg"], np.float64)) * S_WG
    WSM9[:, :, 384:385] = lhsT9(wf2) * S_F12
    WSM9[:, :, 448:449] = lhsT9(wb2) * S_F12
    WSM9 = WSM9[:, TAP_ORDER].astype(f8np)  # [128, 9, 512], tap-reordered
    W0T = np.ascontiguousarray(np.asarray(inputs["conv0_w"], np.float32).T
                               .reshape(12, 128, 8).transpose(1, 0, 2)).astype(np.float16)
    WSM = np.concatenate([
        np.asarray(inputs["conv1_w"], np.float32).T.reshape(-1),
        np.asarray(inputs["conv2_w"], np.float32).T.reshape(-1)]).astype(np.float16)  # [144]
    BSGB = np.concatenate([
        np.stack([inputs["s0_bs"], inputs["s1_bs"], inputs["s2_bs"]]).T * S_H,  # [128,3]
        (1.0 + np.asarray(inputs["s0_bg"], np.float32)).reshape(12, 128).T,     # [128,12]
    ], axis=1).astype(np.float32)                                               # [128,15]
    c0w = np.asarray(inputs["conv0_w"], np.float64)
    c1w = np.asarray(inputs["conv1_w"], np.float64)
    c2w = np.asarray(inputs["conv2_w"], np.float64)
    BIASV = np.zeros((16, 8), np.float32)
    BIASV[0:8, 0] = 1.0 + np.asarray(inputs["s1_bg"], np.float64)
    BIASV[0:16, 1] = 1.0 + np.asarray(inputs["s2_bg"], np.float64)
    BIASV[0:8, 2] = (np.asarray(inputs["b0"], np.float64)
                     + c0w @ np.asarray(inputs["s0_bb"], np.float64))
    BIASV[0:16, 3] = (np.asarray(inputs["b1"], np.float64)
                      + c1w @ np.asarray(inputs["s1_bb"], np.float64))
    BIASV[0:1, 4] = (np.asarray(inputs["b2"], np.float64)
                     + c2w @ np.asarray(inputs["s2_bb"], np.float64))
    BIASV[0:8, 5] = c0w @ (1.0 + np.asarray(inputs["s0_bg"], np.float64))
    BIASV[0:16, 6] = c1w @ (1.0 + np.asarray(inputs["s1_bg"], np.float64))
    BIASV[0:1, 7] = c2w @ (1.0 + np.asarray(inputs["s2_bg"], np.float64))

    shared = dict(ws01=WS01, ws2=WS2, wg=WG, wsm9=WSM9, w0t=W0T,
                  wsm=WSM, bsgb=BSGB, biasv=BIASV)

    in_maps = []
    for core in range(8):
        k, h = core // 2, core % 2
        r0 = HROWS * h
        X = np.ascontiguousarray(
            x_main[k, :, r0:r0 + HROWS, :].reshape(12, 128, 512).transpose(1, 0, 2)
        ).astype(np.float16)
        FT = np.ascontiguousarray(
            f_sem[k].reshape(384, NPOS).T.reshape(8, 128, 384).transpose(1, 0, 2)
        ).astype(np.float16)
        ids_flat = seg[k, ::14, ::14].astype(np.float32).reshape(NPOS)
        IDS = np.ascontiguousarray(ids_flat.reshape(8, 128).T)
        rows = np.arange(r0 - 2, r0 + HROWS + 2)          # 20 sm rows
        valid = (rows >= 0) & (rows < Hp)
        rcl = np.clip(rows, 0, Hp - 1)
        cid = np.empty((SMR, Wp, 4), np.float32)
        cols = np.arange(Wp)
        for t, (dy, dx) in enumerate([(0, 0), (0, 1), (1, 0), (1, 1)]):
            v = seg[k][np.ix_(14 * rcl + 6 + dy, 14 * cols + 6 + dx)].astype(np.float32)
            v[~valid, :] = -1.0
            cid[:, :, t] = v
        CID = np.ascontiguousarray(cid.reshape(5, 128, 4).transpose(1, 0, 2))
        hrows = np.arange(r0 - 1, r0 + HROWS + 1)
        HM = ((hrows >= 0) & (hrows < Hp)).astype(np.float32)
        in_maps.append(dict(shared, x=X, ft=FT, ids=IDS, cid=CID, hmask=HM))
    return in_maps


def kernel(**inputs):
    global _BUILT, LAST_RESULTS
    if _BUILT is None:
        _BUILT = _build_nc()
    nc = _BUILT
    in_maps = _host_prep(inputs)
    trace = bool(os.environ.get("BASS_TRACE"))
    res = run_bass_kernel_spmd(nc, in_maps, list(range(8)), trace=trace)
    LAST_RESULTS = res
    out = np.empty((B, 1, Hp, Wp), np.float32)
    for core in range(8):
        k, h = core // 2, core % 2
        out[k, 0, HROWS * h:HROWS * (h + 1), :] = \
            res.results[core]["out_half"].reshape(HROWS, Wp)
    return out


# revision 33
# speedup vs baseline: 1.1903x; 1.0087x over previous
"""Trainium2 Bass kernel for nn_DinoGazeSpade (segment_reduce + SPADE stack).

Layout: 8 cores; image k = core//2; each core computes rows [16h, 16h+16) of
the 32x32 grid (h = core%2). Cross-core: 3 pairwise AllReduces of LayerNorm
partial stats.

Structure (v2):
  - All 3x3 convs (sm->h, h->gp1, h->C/B folds, h->gp11/gp12) run as fp8e4m3
    DoubleRow matmuls (2 k-tiles per op) with host power-of-2 weight scaling;
    descale folded into the PSUM-read activation / combine scalars.
  - B terms (W @ gp) folded on host into conv weights from h (W@wg), merged
    into the same PSUM group as the C fold -> no gp-dependent projections.
  - LayerNorm stats never touch the PE: bn_stats/aggr (vector) ->
    gpsimd tensor_reduce(axis=C) -> DMA -> AllReduce -> broadcast-DMA of the
    raw sums to all partitions; rsqrt = vector.reciprocal + scalar Sqrt.
  - Scalar engine uses only {Sqrt, Relu, Identity, Copy, Square} => a single
    activation table set, no ACT_TABLE_LOAD ping-pong. softplus = relu(z) +
    rational(2,3)(min(|z|,8)) on the vector engine (max abs err ~7e-4).
  - PE program order: all stat-independent matmuls first (oh, sm, h0..h2,
    C/B folds, gp convs, conv_g+A0 pipelined), A1/A2 at the tail.
  - h-conv halo rows are zeroed by masking the PSUM (f32) before the relu.
"""
import os
import numpy as np
from contextlib import ExitStack

import ml_dtypes

import concourse.bass as bass
import concourse.mybir as mybir
import concourse.tile as tile
from concourse import bacc
from concourse.bass_utils import run_bass_kernel_spmd
from concourse.masks import make_identity

f32 = mybir.dt.float32
f16 = mybir.dt.float16
f8 = mybir.dt.float8e4
AF = mybir.ActivationFunctionType
ALU = mybir.AluOpType
AX = mybir.AxisListType
DR = mybir.MatmulPerfMode.DoubleRow

NSEG = 64
B, Cd, Hp, Wp, H, W, Cm, HID = 4, 384, 32, 32, 448, 448, 1536, 128
NPOS = Hp * Wp          # 1024
HROWS = 16              # rows per core
SMR = HROWS + 4         # sm rows incl 2-halo each side = 20
HR = HROWS + 2          # h rows incl 1-halo each side = 18
SMW = 48                # padded width (row stride must be 16B-aligned for dual-fp8)

# fp8 scale exponents (power of two)
S_SM = 8.0      # sm values
S_WS = 64.0     # ws weights
S_H = 4.0       # h values
S_WG = 64.0     # conv_g / wg1 / wg2 weights
S_F0 = 64.0     # layer-0 C/B fold weights
S_F12 = 256.0   # layer-1/2 C/B fold weights

# softplus tail g(t)=ln(1+exp(-t)) ~ (c0+c1 t+c2 t^2)/(1+d1 t+d2 t^2+d3 t^3)
# fit on t in [0,8] (t clamped at 8; g(8)=3.35e-4), max abs err 3.9e-4
SP_C0, SP_C1, SP_C2 = 0.6934867715618367, -0.17760652420286008, 0.011840728429853564
SP_D1, SP_D2, SP_D3 = 0.477190455932838, 0.1387482411055944, 0.0669674223997194

LAST_RESULTS = None  # set by kernel() for test harness introspection

_BUILT = None

TAPS = [(t // 3, t % 3) for t in range(9)]
# Dual-fp8 moving APs need all outer steps 16B-aligned: pair taps VERTICALLY
# (delta = row stride 48). Pairs (t, t+3) for t in 0..2; taps 6..8 single.
# Weight tensors store taps in order [0,3,1,4,2,5,6,7,8] so pairs and the
# k-tile dim are adjacent.
VPAIRS = [(0, 3), (1, 4), (2, 5)]
VSINGLES = [6, 7, 8]
TAP_ORDER = [0, 3, 1, 4, 2, 5, 6, 7, 8]


def _pair_ap(a, delta):
    """Insert a [delta, 2] k-tile dim as dim 1 of an AP (for DoubleRow rhs)."""
    ap = list(a.ap)
    new = [ap[0], [delta, 2]] + list(ap[1:])
    return bass.AP(a.tensor, a.offset, new)


def _softplus(nc, pool, z, out_tile, p, tag):
    """out = softplus(z) = relu(z) + g(min(|z|,8)); scalar does the relu."""
    ta = pool.tile([p, 512], f32, tag=f"sp_ta{tag}")
    nc.scalar.activation(out=ta, in_=z, func=AF.Abs, bias=0.0)
    t = pool.tile([p, 512], f32, tag=f"sp_t{tag}")
    nc.vector.tensor_scalar(out=t, in0=ta, scalar1=8.0, scalar2=None,
                            op0=ALU.min)
    t2 = pool.tile([p, 512], f32, tag=f"sp_t2{tag}")
    nc.scalar.activation(out=t2, in_=t, func=AF.Square, bias=0.0)
    rl = pool.tile([p, 512], f32, tag=f"sp_rl{tag}")
    nc.scalar.activation(out=rl, in_=z, func=AF.Relu, bias=0.0)
    n1 = pool.tile([p, 512], f32, tag=f"sp_n1{tag}")
    nc.vector.tensor_scalar(out=n1, in0=t, scalar1=SP_C1, scalar2=SP_C0,
                            op0=ALU.mult, op1=ALU.add)
    num = pool.tile([p, 512], f32, tag=f"sp_num{tag}")
    nc.vector.scalar_tensor_tensor(out=num, in0=t2, scalar=SP_C2, in1=n1,
                                   op0=ALU.mult, op1=ALU.add)
    d1p = pool.tile([p, 512], f32, tag=f"sp_d1{tag}")
    nc.vector.tensor_scalar(out=d1p, in0=t, scalar1=SP_D1, scalar2=1.0,
                            op0=ALU.mult, op1=ALU.add)
    q = pool.tile([p, 512], f32, tag=f"sp_q{tag}")
    nc.vector.tensor_scalar(out=q, in0=t, scalar1=SP_D3, scalar2=SP_D2,
                            op0=ALU.mult, op1=ALU.add)
    den = pool.tile([p, 512], f32, tag=f"sp_den{tag}")
    nc.vector.tensor_tensor(out=den, in0=q, in1=t2, op=ALU.mult)
    nc.vector.tensor_tensor(out=den, in0=den, in1=d1p, op=ALU.add)
    rd = pool.tile([p, 512], f32, tag=f"sp_rd{tag}")
    nc.vector.reciprocal_approx_fast(out=rd, in_=den)
    gg = pool.tile([p, 512], f32, tag=f"sp_gg{tag}")
    nc.vector.tensor_tensor(out=gg, in0=num, in1=rd, op=ALU.mult)
    nc.vector.tensor_tensor(out=out_tile, in0=gg, in1=rl, op=ALU.add)


def _build_nc():
    stage = int(os.environ.get("KBISECT", "99"))
    nc = bacc.Bacc("TRN2", num_devices=8)

    # (f32, 0.0) const AP is pre-registered by the framework
    nc.all_engine_barrier()

    # ---------------- DRAM I/O ----------------
    d_x = nc.dram_tensor("x", [128, 12, 512], f16, kind="ExternalInput")
    d_ft = nc.dram_tensor("ft", [128, 8, 384], f16, kind="ExternalInput")
    d_ids = nc.dram_tensor("ids", [128, 8], f32, kind="ExternalInput")
    d_cid = nc.dram_tensor("cid", [128, 5, 4], f32, kind="ExternalInput")
    d_hmask = nc.dram_tensor("hmask", [HR], f32, kind="ExternalInput")
    d_ws01 = nc.dram_tensor("ws01", [128, 3, 9, 2, 128], f8, kind="ExternalInput")
    d_ws2 = nc.dram_tensor("ws2", [128, 3, 9, 128], f8, kind="ExternalInput")
    d_wg = nc.dram_tensor("wg", [128, 12, 9, 128], f8, kind="ExternalInput")
    # wsm9: 8 groups of 64 cols (dual-fp8 w/ windowed moving wants >=64):
    # wf0@0(8) wb0@64(8) wg1@128(8) wf1@192(16) wb1@256(16) wg2@320(16)
    # wf2@384(1) wb2@448(1)
    d_wsm9 = nc.dram_tensor("wsm9", [128, 9, 512], f8, kind="ExternalInput")
    d_w0t = nc.dram_tensor("w0t", [128, 12, 8], f16, kind="ExternalInput")
    d_wsm = nc.dram_tensor("wsm", [144], f16, kind="ExternalInput")  # w1t|w2t
    d_bsgb = nc.dram_tensor("bsgb", [128, 15], f32, kind="ExternalInput")
    # biasv [16,8] cols: gb1 gb2 b0f b1f b2f B0c B1c B2c (each from row 0)
    d_biasv = nc.dram_tensor("biasv", [16, 8], f32, kind="ExternalInput")
    d_out = nc.dram_tensor("out_half", [512], f32, kind="ExternalOutput")

    st_l = [nc.dram_tensor(f"st{i}_l", [2], f32) for i in range(3)]
    st_g = [nc.dram_tensor(f"st{i}_g", [2], f32) for i in range(3)]

    with ExitStack() as ctx:
        tc = ctx.enter_context(tile.TileContext(nc, num_cores=8))
        cpool = ctx.enter_context(tc.tile_pool(name="consts", bufs=1))
        dpool = ctx.enter_context(tc.tile_pool(name="data", bufs=1))
        spool = ctx.enter_context(tc.tile_pool(name="small", bufs=1))
        ps = ctx.enter_context(tc.tile_pool(name="ps", bufs=1, space="PSUM"))

        def MAIN(shape, name):
            return ps.tile(shape, f32, tag="ps_main", bufs=2, name=name)

        def PSA(shape, name):  # psA0 -> psA1 -> psA2
            return ps.tile(shape, f32, tag="ps_a", bufs=1, name=name)

        def PSC(shape, name):  # psC0 -> psC1 -> psC2
            return ps.tile(shape, f32, tag="ps_c", bufs=2, name=name)

        def PSB(shape, name):  # seg sums -> psB0 -> psB1 -> psB2 (bank reuse)
            return ps.tile(shape, f32, tag="ps_sums", bufs=1, name=name)

        def PSG(shape, name, dt=f32):  # gr transposes, psg1, psg2
            return ps.tile(shape, dt, tag="ps_g", bufs=2, name=name)

        # ---- gpsimd constants first (iota gates the OH build) ----
        iot = cpool.tile([128, 64], f32)
        nc.gpsimd.iota(iot, pattern=[[1, 64]], base=0, channel_multiplier=0,
                       allow_small_or_imprecise_dtypes=True)
        ident = cpool.tile([128, 128], f16)
        make_identity(nc, ident)
        ones_col = cpool.tile([128, 1], f32)
        nc.gpsimd.memset(ones_col, 1.0)

        # --------- DMAs: sync queue = big early tensors, in need order ------
        idst = cpool.tile([128, 8], f32)
        nc.sync.dma_start(out=idst, in_=d_ids[:, :])
        cidt = cpool.tile([128, 5, 4], f32)
        nc.sync.dma_start(out=cidt, in_=d_cid[:, :, :])
        xt = dpool.tile([128, 12, 512], f16)
        nc.sync.dma_start(out=xt, in_=d_x[:, :, :])
        feats = dpool.tile([128, 8, 385], f16)
        nc.sync.dma_start(out=feats[:, 0:4, 0:384], in_=d_ft[:, 0:4, :])
        nc.sync.dma_start(out=feats[:, 4:8, 0:384], in_=d_ft[:, 4:8, :])
        ws01_t = cpool.tile([128, 3, 9, 2, 128], f8)
        nc.sync.dma_start(out=ws01_t[:, 0:1], in_=d_ws01[:, 0:1])  # s0_ws first
        ws2_t = cpool.tile([128, 3, 9, 128], f8)
        nc.sync.dma_start(out=ws2_t[:, 0:1], in_=d_ws2[:, 0:1])
        wg_t = cpool.tile([128, 12, 9, 128], f8)
        for g in range(3):
            nc.sync.dma_start(out=wg_t[:, g * 4:(g + 1) * 4],
                              in_=d_wg[:, g * 4:(g + 1) * 4])
        nc.sync.dma_start(out=ws01_t[:, 1:3], in_=d_ws01[:, 1:3])  # s1/s2_ws
        nc.sync.dma_start(out=ws2_t[:, 1:3], in_=d_ws2[:, 1:3])

        # --------- small/later tensors issued from the scalar queue ---------
        wsm9_t = cpool.tile([128, 9, 512], f8)
        nc.scalar.dma_start(out=wsm9_t, in_=d_wsm9[:, :, :])
        w0t_t = cpool.tile([128, 12, 8], f16)
        nc.scalar.dma_start(out=w0t_t, in_=d_w0t[:, :, :])
        bsgb_t = cpool.tile([128, 15], f32)
        nc.scalar.dma_start(out=bsgb_t, in_=d_bsgb[:, :])
        bs_t = bsgb_t[:, 0:3]          # S_H * bs, per conv
        gb0_t = bsgb_t[:, 3:15]        # 1 + bg0
        w1t_t = cpool.tile([8, 16], f16)
        nc.scalar.dma_start(out=w1t_t, in_=d_wsm[0:128].rearrange("(a b) -> a b", b=16))
        w2t_t = cpool.tile([16, 1], f16)
        nc.scalar.dma_start(out=w2t_t, in_=d_wsm[128:144][:, None])
        biasv_t = cpool.tile([16, 8], f32)
        nc.scalar.dma_start(out=biasv_t, in_=d_biasv[:, :])
        gb1_t = biasv_t[0:8, 0:1]
        gb2_t = biasv_t[0:16, 1:2]
        b0f_t = biasv_t[0:8, 2:3]
        b1f_t = biasv_t[0:16, 3:4]
        b2f_t = biasv_t[0:1, 4:5]
        B0c_t = biasv_t[0:8, 5:6]
        B1c_t = biasv_t[0:16, 6:7]
        B2c_t = biasv_t[0:1, 7:8]
        hmask_bc = cpool.tile([128, HR], f32)
        nc.scalar.dma_start(out=hmask_bc, in_=d_hmask[None, :].to_broadcast([128, HR]))

        # ---- engine warmups during the DMA window ----
        warm = cpool.tile([128, 64], f32)
        for _ in range(3):
            nc.vector.memset(warm, 0.0)
        pswarm = MAIN([128, 128], "pswarm")
        for _ in range(6):
            nc.tensor.matmul(pswarm, ident, ident, start=True, stop=True)
        wread = cpool.tile([128, 128], f32)
        nc.scalar.copy(wread, pswarm)

        nc.gpsimd.memset(feats[:, :, 384:385], 1.0)

        # ---------------- segment one-hots (vector) ----------------
        oh_t = dpool.tile([128, 8, 64], f16)
        for qc in range(8):
            nc.vector.tensor_scalar(out=oh_t[:, qc, :], in0=iot,
                                    scalar1=idst[:, qc:qc + 1], scalar2=None,
                                    op0=ALU.is_equal)
        gacc = dpool.tile([128, 5, 64], f16)
        for jc in range(5):
            nc.vector.tensor_scalar(out=gacc[:, jc, :], in0=iot,
                                    scalar1=cidt[:, jc, 0:1], scalar2=None,
                                    op0=ALU.is_equal)
            for corner in range(1, 4):
                nc.vector.scalar_tensor_tensor(
                    out=gacc[:, jc, :], in0=iot,
                    scalar=cidt[:, jc, corner:corner + 1],
                    in1=gacc[:, jc, :], op0=ALU.is_equal, op1=ALU.add)

        # ---------------- segment means avg' [64, 384] ----------------
        psums = ps.tile([64, 385], f32, tag="ps_sums", bufs=1)
        for qc in range(8):
            nc.tensor.matmul(psums, oh_t[:, qc, :], feats[:, qc, :],
                             start=(qc == 0), stop=(qc == 7))
        cnt4 = spool.tile([64, 1], f32, tag="cnt4")
        nc.vector.tensor_scalar(out=cnt4, in0=psums[:, 384:385], scalar1=1.0,
                                scalar2=4.0, op0=ALU.max, op1=ALU.mult)
        recip4 = spool.tile([64, 1], f32, tag="recip4")
        nc.vector.reciprocal(out=recip4, in_=cnt4)
        avg_t = dpool.tile([64, 384], f16)
        nc.vector.tensor_scalar_mul(avg_t, psums[:, 0:384], recip4[:, 0:1])

        # ---------------- G masks -> Gr [64, 640] ----------------
        gr_t = dpool.tile([64, 640], f16)
        for jc in range(5):
            ptr = PSG([64, 128], f"ptr{jc}", dt=f16)
            nc.tensor.transpose(ptr, gacc[:, jc, :], ident)
            nc.scalar.copy(gr_t[:, jc * 128:(jc + 1) * 128], ptr)

        # ---------------- sm (f8, scaled by S_SM) ----------------
        sm_pad = dpool.tile([128, 3, SMR, SMW], f8)
        nc.gpsimd.memset(sm_pad, 0.0)
        for mc in range(3):
            for nch in range(2):
                psm = MAIN([128, 320], f"psm{mc}{nch}")
                nc.tensor.matmul(psm, avg_t[:, mc * 128:(mc + 1) * 128],
                                 gr_t[:, nch * 320:(nch + 1) * 320],
                                 start=True, stop=True)
                nc.scalar.activation(
                    out=sm_pad[:, mc, nch * 10:(nch + 1) * 10, 1:33],
                    in_=psm.rearrange("p (r c) -> p r c", c=32),
                    func=AF.Copy, scale=S_SM)

        # ---------------- h convs (fp8 DoubleRow over kc pairs + taps) ------
        hps = []

        def h_conv(cv):
            hp = dpool.tile([128, HR, SMW], f8, tag=f"hpad{cv}", name=f"hpad{cv}")
            nc.gpsimd.memset(hp, 0.0)
            for nch in range(2):
                psh = MAIN([128, 9 * 32], f"psh{cv}{nch}")
                psh3 = psh.rearrange("p (r c) -> p r c", c=32)
                # kc-pair (0,1) DoubleRow per tap (k-tile delta = plane stride)
                for t, (dy, dx) in enumerate(TAPS):
                    r0 = nch * 9 + dy
                    mv = sm_pad[:, 0, r0:r0 + 9, dx:dx + 32]
                    mv2 = _pair_ap(mv, SMR * SMW)
                    nc.tensor.matmul(psh, ws01_t[:, cv, t, :, :], mv2,
                                     start=(t == 0), stop=False, perf_mode=DR)
                # kc=2: vertical tap-pairs (delta = row stride) + 3 singles
                for i, (ta, tb) in enumerate(VPAIRS):
                    dy, dx = TAPS[ta]
                    r0 = nch * 9 + dy
                    mv = sm_pad[:, 2, r0:r0 + 9, dx:dx + 32]
                    mv2 = _pair_ap(mv, SMW)
                    nc.tensor.matmul(psh, ws2_t[:, cv, 2 * i:2 * i + 2, :], mv2,
                                     start=False, stop=False, perf_mode=DR)
                for j, t in enumerate(VSINGLES):
                    dy, dx = TAPS[t]
                    r0 = nch * 9 + dy
                    nc.tensor.matmul(psh, ws2_t[:, cv, 6 + j, :],
                                     sm_pad[:, 2, r0:r0 + 9, dx:dx + 32],
                                     start=False, stop=(j == 2))
                # zero the out-of-image halo row, then relu -> f8 (scaled S_H)
                nc.vector.tensor_tensor(
                    out=psh3, in0=psh3,
                    in1=hmask_bc[:, nch * 9:(nch + 1) * 9, None].to_broadcast([128, 9, 32]),
                    op=ALU.mult)
                nc.scalar.activation(
                    out=hp[:, nch * 9:(nch + 1) * 9, 1:33], in_=psh3,
                    func=AF.Relu, bias=bs_t[:, cv:cv + 1],
                    scale=S_H / (S_SM * S_WS))
            return hp

        # fold conv: out [64, 512] from hp windows, stationary wsm9 cols
        def fold_conv(hp, cols, pstile):
            for i, (ta, tb) in enumerate(VPAIRS):
                dy, dx = TAPS[ta]
                mv2 = _pair_ap(hp[:, dy:dy + 16, dx:dx + 32], SMW)
                nc.tensor.matmul(pstile, wsm9_t[:, 2 * i:2 * i + 2, cols], mv2,
                                 start=(i == 0), stop=False, perf_mode=DR)
            for j, t in enumerate(VSINGLES):
                dy, dx = TAPS[t]
                nc.tensor.matmul(pstile, wsm9_t[:, 6 + j, cols],
                                 hp[:, dy:dy + 16, dx:dx + 32],
                                 start=False, stop=(j == 2))

        # ------------- LN0 partial stats (vector; x lands early now) -------
        bno0 = dpool.tile([128, 12, 6], f32)
        for kc in range(12):
            nc.vector.bn_stats(out=bno0[:, kc, :], in_=xt[:, kc, :])
        mv0 = spool.tile([128, 2], f32, tag="mv0")
        nc.vector.bn_aggr(out=mv0, in_=bno0)
        mvE0 = spool.tile([128, 2], f32, tag="mvE0")
        nc.vector.tensor_tensor(out=mvE0[:, 0:1], in0=mv0[:, 0:1],
                                in1=mv0[:, 0:1], op=ALU.mult)
        nc.vector.tensor_tensor(out=mvE0[:, 1:2], in0=mv0[:, 1:2],
                                in1=mvE0[:, 0:1], op=ALU.add)
        nc.vector.tensor_scalar(out=mvE0[:, 0:1], in0=mv0[:, 0:1],
                                scalar1=1.0, scalar2=None, op0=ALU.mult)

        h0p = h_conv(0)
        psC0 = PSC([64, 512], "psC0")
        fold_conv(h0p, slice(0, 64), psC0)
        psB0 = PSB([64, 512], "psB0")
        fold_conv(h0p, slice(64, 128), psB0)

        # partition-reduce (PE ones-matmul at an idle point) -> DRAM ->
        # AllReduce -> broadcast the raw sums back to all partitions
        def ln_stats(i, mvE, p_out, bc_tile):
            p_in = mvE.shape[0]
            pst = PSG([2, 1], f"pst{i}")
            nc.tensor.matmul(pst, mvE, ones_col[0:p_in, :],
                             start=True, stop=True)
            st_sb = spool.tile([2, 1], f32, tag=f"st_sb{i}")
            nc.vector.tensor_copy(out=st_sb, in_=pst)
            nc.gpsimd.dma_start(out=st_l[i][:], in_=st_sb[0:2, 0:1])
            nc.gpsimd.collective_compute(
                "AllReduce", ALU.add,
                replica_groups=[[0, 1], [2, 3], [4, 5], [6, 7]],
                ins=[st_l[i][:]], outs=[st_g[i][:]],
            )
            nc.gpsimd.dma_start(out=bc_tile,
                                in_=st_g[i][None, :].to_broadcast([p_out, 2]))

        sums0 = spool.tile([8, 2], f32, tag="sums0")

        # ---------------- conv_g + A0 pipeline ----------------
        gp1 = dpool.tile([128, 12, 512], f16)
        xg = dpool.tile([128, 12, 512], f16)
        psA0 = PSA([8, 512], "psA0")

        def conv_g_chunk(kc):
            psg = MAIN([128, 512], f"psg{kc}")
            for i, (ta, tb) in enumerate(VPAIRS):
                dy, dx = TAPS[ta]
                mv2 = _pair_ap(h0p[:, dy:dy + 16, dx:dx + 32], SMW)
                nc.tensor.matmul(psg, wg_t[:, kc, 2 * i:2 * i + 2, :], mv2,
                                 start=(i == 0), stop=False, perf_mode=DR)
            for j, t in enumerate(VSINGLES):
                dy, dx = TAPS[t]
                nc.tensor.matmul(psg, wg_t[:, kc, 6 + j, :],
                                 h0p[:, dy:dy + 16, dx:dx + 32],
                                 start=False, stop=(j == 2))
            nc.scalar.activation(out=gp1[:, kc, :], in_=psg, func=AF.Identity,
                                 bias=gb0_t[:, kc:kc + 1], scale=1.0 / (S_WG * S_H))
            nc.gpsimd.tensor_tensor(out=xg[:, kc, :], in0=xt[:, kc, :],
                                     in1=gp1[:, kc, :], op=ALU.mult)

        def a0_mm(kc):
            nc.tensor.matmul(psA0, w0t_t[:, kc, :], xg[:, kc, :],
                             start=(kc == 0), stop=(kc == 11))

        for kc in range(12):
            conv_g_chunk(kc)
            if kc >= 2:
                a0_mm(kc - 2)
            if kc == 2:
                # LN0 stats round-trip launches while conv_g still runs
                ln_stats(0, mvE0, 8, sums0)
        a0_mm(10)
        a0_mm(11)

        # ---------------- SPADE1/2 convs (all stat-independent) -------------
        # psB1/psC2/psB2 folds are emitted later (PE tail): they reuse PSUM
        # banks whose previous tenants die only at the z0/z1 combines.
        h1p = h_conv(1)
        psg1 = PSG([64, 512], "psg1")
        fold_conv(h1p, slice(128, 192), psg1)
        gp11 = spool.tile([8, 512], f16, tag="gp11")
        nc.scalar.activation(out=gp11, in_=psg1[0:8, :], func=AF.Identity,
                             bias=gb1_t, scale=1.0 / (S_WG * S_H))
        psC1 = PSC([64, 512], "psC1")
        fold_conv(h1p, slice(192, 256), psC1)
        h2p = h_conv(2)
        psg2 = PSG([64, 512], "psg2")
        fold_conv(h2p, slice(320, 384), psg2)
        gp12 = spool.tile([16, 512], f16, tag="gp12")
        nc.scalar.activation(out=gp12, in_=psg2[0:16, :], func=AF.Identity,
                             bias=gb2_t, scale=1.0 / (S_WG * S_H))

        # ---------------- stats math + combine helpers ----------------
        def ln_finish(i, sums, p, n_inv, work_tag):
            """sums [p,2] broadcast raw sums -> work [p,4]: mu, r, -mu*r."""
            w = spool.tile([p, 6], f32, tag=work_tag)
            nc.vector.tensor_scalar_mul(w[:, 0:2], sums, n_inv)   # mu, E
            nc.vector.tensor_tensor(out=w[:, 2:3], in0=w[:, 0:1],
                                    in1=w[:, 0:1], op=ALU.mult)   # mu^2
            nc.vector.scalar_tensor_tensor(out=w[:, 3:4], in0=w[:, 2:3],
                                           scalar=-1.0, in1=w[:, 1:2],
                                           op0=ALU.mult, op1=ALU.add)  # var
            nc.vector.tensor_scalar(out=w[:, 3:4], in0=w[:, 3:4],
                                    scalar1=1.0, scalar2=1e-12,
                                    op0=ALU.mult, op1=ALU.add)
            nc.vector.reciprocal_approx_fast(out=w[:, 4:5], in_=w[:, 3:4])
            nc.scalar.sqrt(w[:, 5:6], w[:, 4:5])                  # r
            nc.vector.tensor_tensor(out=w[:, 2:3], in0=w[:, 0:1],
                                    in1=w[:, 5:6], op=ALU.mult)   # mu*r
            nc.vector.tensor_scalar_mul(w[:, 3:4], w[:, 2:3], -1.0)  # -mu*r
            return w  # r = w[:,5:6], negmur = w[:,3:4]

        def combine(psA, psC, psB, w, Bc, bf, sCB, p, tag):
            """z = r*A + sCB*((-mu r)*Bv + Cv) + (bf + (-mu r)*Bc)."""
            bias_dyn = spool.tile([p, 1], f32, tag=f"bd{tag}")
            nc.vector.scalar_tensor_tensor(out=bias_dyn, in0=Bc,
                                           scalar=w[:, 3:4], in1=bf,
                                           op0=ALU.mult, op1=ALU.add)
            tb = dpool.tile([p, 512], f32, tag=f"tb{tag}")
            nc.vector.tensor_scalar_mul(tb, psB, w[:, 3:4])
            t1 = dpool.tile([p, 512], f32, tag=f"t1{tag}")
            nc.vector.tensor_tensor(out=t1, in0=tb, in1=psC, op=ALU.add)
            t2 = dpool.tile([p, 512], f32, tag=f"t2{tag}")
            nc.vector.tensor_scalar(out=t2, in0=t1, scalar1=sCB,
                                    scalar2=bias_dyn, op0=ALU.mult, op1=ALU.add)
            z = dpool.tile([p, 512], f32, tag=f"z{tag}")
            nc.vector.scalar_tensor_tensor(out=z, in0=psA, scalar=w[:, 5:6],
                                           in1=t2, op0=ALU.mult, op1=ALU.add)
            return z

        # ---------------- layer 0 tail ----------------
        w0 = ln_finish(0, sums0, 8, 1.0 / 256.0, "w0")
        z0 = combine(psA0, psC0[0:8, :], psB0[0:8, :], w0, B0c_t, b0f_t,
                     1.0 / (S_F0 * S_H), 8, "0")
        out0 = dpool.tile([8, 512], f32)
        _softplus(nc, dpool, z0, out0, 8, "0")

        # og1 first (only needs out0), then LN1 stats, then PE tail folds
        og1 = spool.tile([8, 512], f16, tag="og1")
        nc.vector.tensor_tensor(out=og1, in0=out0, in1=gp11, op=ALU.mult)
        bno1 = spool.tile([8, 1, 6], f32, tag="bno1")
        nc.vector.bn_stats(out=bno1[:, 0, :], in_=out0)
        mv1 = spool.tile([8, 2], f32, tag="mv1")
        nc.vector.bn_aggr(out=mv1, in_=bno1)
        mvE1 = spool.tile([8, 2], f32, tag="mvE1")
        nc.vector.tensor_tensor(out=mvE1[:, 0:1], in0=mv1[:, 0:1],
                                in1=mv1[:, 0:1], op=ALU.mult)
        nc.vector.tensor_tensor(out=mvE1[:, 1:2], in0=mv1[:, 1:2],
                                in1=mvE1[:, 0:1], op=ALU.add)
        nc.vector.tensor_scalar(out=mvE1[:, 0:1], in0=mv1[:, 0:1],
                                scalar1=1.0, scalar2=None, op0=ALU.mult)

        psB1 = PSB([64, 512], "psB1")
        fold_conv(h1p, slice(256, 320), psB1)
        psC2 = PSC([64, 512], "psC2")
        fold_conv(h2p, slice(384, 448), psC2)
        psA1 = PSA([16, 512], "psA1")
        nc.tensor.matmul(psA1, w1t_t, og1, start=True, stop=True)
        sums1 = spool.tile([16, 2], f32, tag="sums1")
        ln_stats(1, mvE1, 16, sums1)

        # ---------------- layer 1 tail ----------------
        w1 = ln_finish(1, sums1, 16, 1.0 / 16.0, "w1")
        z1 = combine(psA1, psC1[0:16, :], psB1[0:16, :], w1, B1c_t, b1f_t,
                     1.0 / (S_F12 * S_H), 16, "1")
        out1 = dpool.tile([16, 512], f32)
        _softplus(nc, dpool, z1, out1, 16, "1")

        # og2 first, then LN2 stats; PE: psB2 fold, A2, pst2
        og2 = spool.tile([16, 512], f16, tag="og2")
        nc.vector.tensor_tensor(out=og2, in0=out1, in1=gp12, op=ALU.mult)
        bno2 = spool.tile([16, 1, 6], f32, tag="bno2")
        nc.vector.bn_stats(out=bno2[:, 0, :], in_=out1)
        mv2 = spool.tile([16, 2], f32, tag="mv2")
        nc.vector.bn_aggr(out=mv2, in_=bno2)
        mvE2 = spool.tile([16, 2], f32, tag="mvE2")
        nc.vector.tensor_tensor(out=mvE2[:, 0:1], in0=mv2[:, 0:1],
                                in1=mv2[:, 0:1], op=ALU.mult)
        nc.vector.tensor_tensor(out=mvE2[:, 1:2], in0=mv2[:, 1:2],
                                in1=mvE2[:, 0:1], op=ALU.add)
        nc.vector.tensor_scalar(out=mvE2[:, 0:1], in0=mv2[:, 0:1],
                                scalar1=1.0, scalar2=None, op0=ALU.mult)

        psB2 = PSB([64, 512], "psB2")
        fold_conv(h2p, slice(448, 512), psB2)
        psA2 = PSA([1, 512], "psA2")
        nc.tensor.matmul(psA2, w2t_t, og2, start=True, stop=True)
        sums2 = spool.tile([1, 2], f32, tag="sums2")
        ln_stats(2, mvE2, 1, sums2)

        # ---------------- layer 2 tail -> output ----------------
        w2 = ln_finish(2, sums2, 1, 1.0 / 32.0, "w2")
        z2 = combine(psA2, psC2[0:1, :], psB2[0:1, :], w2, B2c_t, b2f_t,
                     1.0 / (S_F12 * S_H), 1, "2")
        final = dpool.tile([1, 512], f32)
        _softplus(nc, dpool, z2, final, 1, "2")
        nc.scalar.dma_start(out=d_out[:], in_=final[0:1, :])

    nc.compile()
    return nc


def _host_prep(inputs):
    """Build per-core in_maps (host work: slicing, layout, small weight folds)."""
    x_main = np.asarray(inputs["x_main"], np.float32)
    f_sem = np.asarray(inputs["f_sem"], np.float32)
    seg = np.asarray(inputs["seg_mask"])
    f8np = ml_dtypes.float8_e4m3

    def lhsT9(w):  # [O, I, 3, 3] -> [I, 9, O]
        return np.ascontiguousarray(w.transpose(1, 2, 3, 0).reshape(w.shape[1], 9, w.shape[0]))

    ws_stack = np.stack([inputs["s0_ws"], inputs["s1_ws"], inputs["s2_ws"]])  # [3,128,384,3,3]
    ws_r = ws_stack.reshape(3, 128, 3, 128, 3, 3)          # cv, o, kc, i, ky, kx
    ws_full = (ws_r.transpose(3, 0, 2, 4, 5, 1)
               .reshape(128, 3, 3, 9, 128) * S_WS)         # [i, cv, kc, tap, o]
    WS01 = np.ascontiguousarray(
        ws_full[:, :, 0:2].transpose(0, 1, 3, 2, 4)).astype(f8np)  # [128,3,9,2,128]
    WS2 = np.ascontiguousarray(ws_full[:, :, 2][:, :, TAP_ORDER]).astype(f8np)
    wg0 = np.asarray(inputs["s0_wg"], np.float32)          # [1536, 128, 3, 3]
    WG = np.ascontiguousarray(
        (wg0.reshape(12, 128, 128, 3, 3).transpose(2, 0, 3, 4, 1)
         .reshape(128, 12, 9, 128) * S_WG)[:, :, TAP_ORDER]).astype(f8np)
    wf0 = np.einsum("oc,cikl->oikl", np.asarray(inputs["conv0_w"], np.float64),
                    np.asarray(inputs["s0_wb"], np.float64))
    wb0 = np.einsum("oc,cikl->oikl", np.asarray(inputs["conv0_w"], np.float64),
                    np.asarray(inputs["s0_wg"], np.float64))
    wf1 = np.einsum("oc,cikl->oikl", np.asarray(inputs["conv1_w"], np.float64),
                    np.asarray(inputs["s1_wb"], np.float64))
    wb1 = np.einsum("oc,cikl->oikl", np.asarray(inputs["conv1_w"], np.float64),
                    np.asarray(inputs["s1_wg"], np.float64))
    wf2 = np.einsum("oc,cikl->oikl", np.asarray(inputs["conv2_w"], np.float64),
                    np.asarray(inputs["s2_wb"], np.float64))
    wb2 = np.einsum("oc,cikl->oikl", np.asarray(inputs["conv2_w"], np.float64),
                    np.asarray(inputs["s2_wg"], np.float64))
    WSM9 = np.zeros((128, 9, 512), np.float64)
    WSM9[:, :, 0:8] = lhsT9(wf0) * S_F0
    WSM9[:, :, 64:72] = lhsT9(wb0) * S_F0
    WSM9[:, :, 128:136] = lhsT9(np.asarray(inputs["s1_wg"], np.float64)) * S_WG
    WSM9[:, :, 192:208] = lhsT9(wf1) * S_F12
    WSM9[:, :, 256:272] = lhsT9(wb1) * S_F12
    WSM9[:, :, 320:336] = lhsT9(np.asarray(inputs["s2_wg"], np.float64)) * S_WG
    WSM9[:, :, 384:385] = lhsT9(wf2) * S_F12
    WSM9[:, :, 448:449] = lhsT9(wb2) * S_F12
    WSM9 = WSM9[:, TAP_ORDER].astype(f8np)  # [128, 9, 512], tap-reordered
    W0T = np.ascontiguousarray(np.asarray(inputs["conv0_w"], np.float32).T
                               .reshape(12, 128, 8).transpose(1, 0, 2)).astype(np.float16)
    WSM = np.concatenate([
        np.asarray(inputs["conv1_w"], np.float32).T.reshape(-1),
        np.asarray(inputs["conv2_w"], np.float32).T.reshape(-1)]).astype(np.float16)  # [144]
    BSGB = np.concatenate([
        np.stack([inputs["s0_bs"], inputs["s1_bs"], inputs["s2_bs"]]).T * S_H,  # [128,3]
        (1.0 + np.asarray(inputs["s0_bg"], np.float32)).reshape(12, 128).T,     # [128,12]
    ], axis=1).astype(np.float32)                                               # [128,15]
    c0w = np.asarray(inputs["conv0_w"], np.float64)
    c1w = np.asarray(inputs["conv1_w"], np.float64)
    c2w = np.asarray(inputs["conv2_w"], np.float64)
    BIASV = np.zeros((16, 8), np.float32)
    BIASV[0:8, 0] = 1.0 + np.asarray(inputs["s1_bg"], np.float64)
    BIASV[0:16, 1] = 1.0 + np.asarray(inputs["s2_bg"], np.float64)
    BIASV[0:8, 2] = (np.asarray(inputs["b0"], np.float64)
                     + c0w @ np.asarray(inputs["s0_bb"], np.float64))
    BIASV[0:16, 3] = (np.asarray(inputs["b1"], np.float64)
                      + c1w @ np.asarray(inputs["s1_bb"], np.float64))
    BIASV[0:1, 4] = (np.asarray(inputs["b2"], np.float64)
                     + c2w @ np.asarray(inputs["s2_bb"], np.float64))
    BIASV[0:8, 5] = c0w @ (1.0 + np.asarray(inputs["s0_bg"], np.float64))
    BIASV[0:16, 6] = c1w @ (1.0 + np.asarray(inputs["s1_bg"], np.float64))
    BIASV[0:1, 7] = c2w @ (1.0 + np.asarray(inputs["s2_bg"], np.float64))

    shared = dict(ws01=WS01, ws2=WS2, wg=WG, wsm9=WSM9, w0t=W0T,
                  wsm=WSM, bsgb=BSGB, biasv=BIASV)

    in_maps = []
    for core in range(8):
        k, h = core // 2, core % 2
        r0 = HROWS * h
        X = np.ascontiguousarray(
            x_main[k, :, r0:r0 + HROWS, :].reshape(12, 128, 512).transpose(1, 0, 2)
        ).astype(np.float16)
        FT = np.ascontiguousarray(
            f_sem[k].reshape(384, NPOS).T.reshape(8, 128, 384).transpose(1, 0, 2)
        ).astype(np.float16)
        ids_flat = seg[k, ::14, ::14].astype(np.float32).reshape(NPOS)
        IDS = np.ascontiguousarray(ids_flat.reshape(8, 128).T)
        rows = np.arange(r0 - 2, r0 + HROWS + 2)          # 20 sm rows
        valid = (rows >= 0) & (rows < Hp)
        rcl = np.clip(rows, 0, Hp - 1)
        cid = np.empty((SMR, Wp, 4), np.float32)
        cols = np.arange(Wp)
        for t, (dy, dx) in enumerate([(0, 0), (0, 1), (1, 0), (1, 1)]):
            v = seg[k][np.ix_(14 * rcl + 6 + dy, 14 * cols + 6 + dx)].astype(np.float32)
            v[~valid, :] = -1.0
            cid[:, :, t] = v
        CID = np.ascontiguousarray(cid.reshape(5, 128, 4).transpose(1, 0, 2))
        hrows = np.arange(r0 - 1, r0 + HROWS + 1)
        HM = ((hrows >= 0) & (hrows < Hp)).astype(np.float32)
        in_maps.append(dict(shared, x=X, ft=FT, ids=IDS, cid=CID, hmask=HM))
    return in_maps


def kernel(**inputs):
    global _BUILT, LAST_RESULTS
    if _BUILT is None:
        _BUILT = _build_nc()
    nc = _BUILT
    in_maps = _host_prep(inputs)
    trace = bool(os.environ.get("BASS_TRACE"))
    res = run_bass_kernel_spmd(nc, in_maps, list(range(8)), trace=trace)
    LAST_RESULTS = res
    out = np.empty((B, 1, Hp, Wp), np.float32)
    for core in range(8):
        k, h = core // 2, core % 2
        out[k, 0, HROWS * h:HROWS * (h + 1), :] = \
            res.results[core]["out_half"].reshape(HROWS, Wp)
    return out


# revision 34
# speedup vs baseline: 1.2819x; 1.0769x over previous
"""Trainium2 Bass kernel for nn_DinoGazeSpade (segment_reduce + SPADE stack).

Layout: 8 cores; image k = core//2; each core computes rows [16h, 16h+16) of
the 32x32 grid (h = core%2). Cross-core: 3 pairwise AllReduces of LayerNorm
partial stats.

Structure (v2):
  - All 3x3 convs (sm->h, h->gp1, h->C/B folds, h->gp11/gp12) run as fp8e4m3
    DoubleRow matmuls (2 k-tiles per op) with host power-of-2 weight scaling;
    descale folded into the PSUM-read activation / combine scalars.
  - B terms (W @ gp) folded on host into conv weights from h (W@wg), merged
    into the same PSUM group as the C fold -> no gp-dependent projections.
  - LayerNorm stats never touch the PE: bn_stats/aggr (vector) ->
    gpsimd tensor_reduce(axis=C) -> DMA -> AllReduce -> broadcast-DMA of the
    raw sums to all partitions; rsqrt = vector.reciprocal + scalar Sqrt.
  - Scalar engine uses only {Sqrt, Relu, Identity, Copy, Square} => a single
    activation table set, no ACT_TABLE_LOAD ping-pong. softplus = relu(z) +
    rational(2,3)(min(|z|,8)) on the vector engine (max abs err ~7e-4).
  - PE program order: all stat-independent matmuls first (oh, sm, h0..h2,
    C/B folds, gp convs, conv_g+A0 pipelined), A1/A2 at the tail.
  - h-conv halo rows are zeroed by masking the PSUM (f32) before the relu.
"""
import os
import numpy as np
from contextlib import ExitStack

import ml_dtypes

import concourse.bass as bass
import concourse.mybir as mybir
import concourse.tile as tile
from concourse import bacc
from concourse.bass_utils import run_bass_kernel_spmd
from concourse.masks import make_identity

f32 = mybir.dt.float32
f16 = mybir.dt.float16
f8 = mybir.dt.float8e4
AF = mybir.ActivationFunctionType
ALU = mybir.AluOpType
AX = mybir.AxisListType
DR = mybir.MatmulPerfMode.DoubleRow

NSEG = 64
B, Cd, Hp, Wp, H, W, Cm, HID = 4, 384, 32, 32, 448, 448, 1536, 128
NPOS = Hp * Wp          # 1024
HROWS = 16              # rows per core
SMR = HROWS + 4         # sm rows incl 2-halo each side = 20
HR = HROWS + 2          # h rows incl 1-halo each side = 18
SMW = 48                # padded width (row stride must be 16B-aligned for dual-fp8)

# fp8 scale exponents (power of two)
S_SM = 8.0      # sm values
S_WS = 64.0     # ws weights
S_H = 4.0       # h values
S_WG = 64.0     # conv_g / wg1 / wg2 weights
S_F0 = 64.0     # layer-0 C/B fold weights
S_F12 = 256.0   # layer-1/2 C/B fold weights

# softplus tail g(t)=ln(1+exp(-t)) ~ (c0+c1 t+c2 t^2)/(1+d1 t+d2 t^2)
# fit on t in [0,8] (t clamped at 8; g(8)=3.35e-4), max abs err 2.1e-3
SP_C0, SP_C1, SP_C2 = 0.6912142642393667, -0.20836163680771763, 0.015927195001436325
SP_D1, SP_D2 = 0.37255004800107144, 0.22851532868409638

LAST_RESULTS = None  # set by kernel() for test harness introspection

_BUILT = None

TAPS = [(t // 3, t % 3) for t in range(9)]
# Dual-fp8 moving APs need all outer steps 16B-aligned: pair taps VERTICALLY
# (delta = row stride 48). Pairs (t, t+3) for t in 0..2; taps 6..8 single.
# Weight tensors store taps in order [0,3,1,4,2,5,6,7,8] so pairs and the
# k-tile dim are adjacent.
VPAIRS = [(0, 3), (1, 4), (2, 5)]
VSINGLES = [6, 7, 8]
TAP_ORDER = [0, 3, 1, 4, 2, 5, 6, 7, 8]


def _pair_ap(a, delta):
    """Insert a [delta, 2] k-tile dim as dim 1 of an AP (for DoubleRow rhs)."""
    ap = list(a.ap)
    new = [ap[0], [delta, 2]] + list(ap[1:])
    return bass.AP(a.tensor, a.offset, new)


def _softplus(nc, pool, z, out_tile, p, tag):
    """out = softplus(z) = relu(z) + g(min(|z|,8)); scalar does the relu."""
    ta = pool.tile([p, 512], f32, tag=f"sp_ta{tag}")
    nc.scalar.activation(out=ta, in_=z, func=AF.Abs, bias=0.0)
    t = pool.tile([p, 512], f32, tag=f"sp_t{tag}")
    nc.vector.tensor_scalar(out=t, in0=ta, scalar1=8.0, scalar2=None,
                            op0=ALU.min)
    t2 = pool.tile([p, 512], f32, tag=f"sp_t2{tag}")
    nc.scalar.activation(out=t2, in_=t, func=AF.Square, bias=0.0)
    rl = pool.tile([p, 512], f32, tag=f"sp_rl{tag}")
    nc.scalar.activation(out=rl, in_=z, func=AF.Relu, bias=0.0)
    n1 = pool.tile([p, 512], f32, tag=f"sp_n1{tag}")
    nc.vector.tensor_scalar(out=n1, in0=t, scalar1=SP_C1, scalar2=SP_C0,
                            op0=ALU.mult, op1=ALU.add)
    num = pool.tile([p, 512], f32, tag=f"sp_num{tag}")
    nc.vector.scalar_tensor_tensor(out=num, in0=t2, scalar=SP_C2, in1=n1,
                                   op0=ALU.mult, op1=ALU.add)
    d1p = pool.tile([p, 512], f32, tag=f"sp_d1{tag}")
    nc.vector.tensor_scalar(out=d1p, in0=t, scalar1=SP_D1, scalar2=1.0,
                            op0=ALU.mult, op1=ALU.add)
    den = pool.tile([p, 512], f32, tag=f"sp_den{tag}")
    nc.vector.scalar_tensor_tensor(out=den, in0=t2, scalar=SP_D2, in1=d1p,
                                   op0=ALU.mult, op1=ALU.add)
    rd = pool.tile([p, 512], f32, tag=f"sp_rd{tag}")
    nc.vector.reciprocal_approx_fast(out=rd, in_=den)
    gg = pool.tile([p, 512], f32, tag=f"sp_gg{tag}")
    nc.vector.tensor_tensor(out=gg, in0=num, in1=rd, op=ALU.mult)
    nc.vector.tensor_tensor(out=out_tile, in0=gg, in1=rl, op=ALU.add)


def _build_nc():
    stage = int(os.environ.get("KBISECT", "99"))
    nc = bacc.Bacc("TRN2", num_devices=8)

    # (f32, 0.0) const AP is pre-registered by the framework
    nc.all_engine_barrier()

    # ---------------- DRAM I/O ----------------
    d_x = nc.dram_tensor("x", [128, 12, 512], f16, kind="ExternalInput")
    d_ft = nc.dram_tensor("ft", [128, 8, 384], f16, kind="ExternalInput")
    d_ids = nc.dram_tensor("ids", [128, 8], f32, kind="ExternalInput")
    d_cid = nc.dram_tensor("cid", [128, 5, 4], f32, kind="ExternalInput")
    d_hmask = nc.dram_tensor("hmask", [HR], f32, kind="ExternalInput")
    d_ws01 = nc.dram_tensor("ws01", [128, 3, 9, 2, 128], f8, kind="ExternalInput")
    d_ws2 = nc.dram_tensor("ws2", [128, 3, 9, 128], f8, kind="ExternalInput")
    d_wg = nc.dram_tensor("wg", [128, 12, 9, 128], f8, kind="ExternalInput")
    # wsm9: 8 groups of 64 cols (dual-fp8 w/ windowed moving wants >=64):
    # wf0@0(8) wb0@64(8) wg1@128(8) wf1@192(16) wb1@256(16) wg2@320(16)
    # wf2@384(1) wb2@448(1)
    d_wsm9 = nc.dram_tensor("wsm9", [128, 9, 512], f8, kind="ExternalInput")
    d_w0t = nc.dram_tensor("w0t", [128, 12, 8], f16, kind="ExternalInput")
    d_wsm = nc.dram_tensor("wsm", [144], f16, kind="ExternalInput")  # w1t|w2t
    d_bsgb = nc.dram_tensor("bsgb", [128, 15], f32, kind="ExternalInput")
    # biasv [16,8] cols: gb1 gb2 b0f b1f b2f B0c B1c B2c (each from row 0)
    d_biasv = nc.dram_tensor("biasv", [16, 8], f32, kind="ExternalInput")
    d_out = nc.dram_tensor("out_half", [512], f32, kind="ExternalOutput")

    st_l = [nc.dram_tensor(f"st{i}_l", [2], f32) for i in range(3)]
    st_g = [nc.dram_tensor(f"st{i}_g", [2], f32) for i in range(3)]

    with ExitStack() as ctx:
        tc = ctx.enter_context(tile.TileContext(nc, num_cores=8))
        cpool = ctx.enter_context(tc.tile_pool(name="consts", bufs=1))
        dpool = ctx.enter_context(tc.tile_pool(name="data", bufs=1))
        spool = ctx.enter_context(tc.tile_pool(name="small", bufs=1))
        ps = ctx.enter_context(tc.tile_pool(name="ps", bufs=1, space="PSUM"))

        def MAIN(shape, name):
            return ps.tile(shape, f32, tag="ps_main", bufs=2, name=name)

        def PSA(shape, name):  # psA0 -> psA1 -> psA2
            return ps.tile(shape, f32, tag="ps_a", bufs=1, name=name)

        def PSC(shape, name):  # psC0 -> psC1 -> psC2
            return ps.tile(shape, f32, tag="ps_c", bufs=2, name=name)

        def PSB(shape, name):  # seg sums -> psB0 -> psB1 -> psB2 (bank reuse)
            return ps.tile(shape, f32, tag="ps_sums", bufs=1, name=name)

        def PSG(shape, name, dt=f32):  # gr transposes, psg1, psg2
            return ps.tile(shape, dt, tag="ps_g", bufs=2, name=name)

        # ---- gpsimd constants first (iota gates the OH build) ----
        iot = cpool.tile([128, 64], f32)
        nc.gpsimd.iota(iot, pattern=[[1, 64]], base=0, channel_multiplier=0,
                       allow_small_or_imprecise_dtypes=True)
        ident = cpool.tile([128, 128], f16)
        make_identity(nc, ident)
        ones_col = cpool.tile([128, 1], f32)
        nc.gpsimd.memset(ones_col, 1.0)

        # --------- DMAs: sync queue = big early tensors, in need order ------
        idst = cpool.tile([128, 8], f32)
        nc.sync.dma_start(out=idst, in_=d_ids[:, :])
        cidt = cpool.tile([128, 5, 4], f32)
        nc.sync.dma_start(out=cidt, in_=d_cid[:, :, :])
        xt = dpool.tile([128, 12, 512], f16)
        nc.sync.dma_start(out=xt, in_=d_x[:, :, :])
        feats = dpool.tile([128, 8, 385], f16)
        nc.sync.dma_start(out=feats[:, 0:4, 0:384], in_=d_ft[:, 0:4, :])
        nc.sync.dma_start(out=feats[:, 4:8, 0:384], in_=d_ft[:, 4:8, :])
        ws01_t = cpool.tile([128, 3, 9, 2, 128], f8)
        nc.sync.dma_start(out=ws01_t[:, 0:1], in_=d_ws01[:, 0:1])  # s0_ws first
        ws2_t = cpool.tile([128, 3, 9, 128], f8)
        nc.sync.dma_start(out=ws2_t[:, 0:1], in_=d_ws2[:, 0:1])
        wg_t = cpool.tile([128, 12, 9, 128], f8)
        for g in range(3):
            nc.sync.dma_start(out=wg_t[:, g * 4:(g + 1) * 4],
                              in_=d_wg[:, g * 4:(g + 1) * 4])
        nc.sync.dma_start(out=ws01_t[:, 1:3], in_=d_ws01[:, 1:3])  # s1/s2_ws
        nc.sync.dma_start(out=ws2_t[:, 1:3], in_=d_ws2[:, 1:3])

        # --------- small/later tensors issued from the scalar queue ---------
        wsm9_t = cpool.tile([128, 9, 512], f8)
        nc.scalar.dma_start(out=wsm9_t, in_=d_wsm9[:, :, :])
        w0t_t = cpool.tile([128, 12, 8], f16)
        nc.scalar.dma_start(out=w0t_t, in_=d_w0t[:, :, :])
        bsgb_t = cpool.tile([128, 15], f32)
        nc.scalar.dma_start(out=bsgb_t, in_=d_bsgb[:, :])
        bs_t = bsgb_t[:, 0:3]          # S_H * bs, per conv
        gb0_t = bsgb_t[:, 3:15]        # 1 + bg0
        w1t_t = cpool.tile([8, 16], f16)
        nc.scalar.dma_start(out=w1t_t, in_=d_wsm[0:128].rearrange("(a b) -> a b", b=16))
        w2t_t = cpool.tile([16, 1], f16)
        nc.scalar.dma_start(out=w2t_t, in_=d_wsm[128:144][:, None])
        biasv_t = cpool.tile([16, 8], f32)
        nc.scalar.dma_start(out=biasv_t, in_=d_biasv[:, :])
        gb1_t = biasv_t[0:8, 0:1]
        gb2_t = biasv_t[0:16, 1:2]
        b0f_t = biasv_t[0:8, 2:3]
        b1f_t = biasv_t[0:16, 3:4]
        b2f_t = biasv_t[0:1, 4:5]
        B0c_t = biasv_t[0:8, 5:6]
        B1c_t = biasv_t[0:16, 6:7]
        B2c_t = biasv_t[0:1, 7:8]
        hmask_bc = cpool.tile([128, HR], f32)
        nc.scalar.dma_start(out=hmask_bc, in_=d_hmask[None, :].to_broadcast([128, HR]))

        # ---- engine warmups during the DMA window ----
        warm = cpool.tile([128, 64], f32)
        for _ in range(3):
            nc.vector.memset(warm, 0.0)
        pswarm = MAIN([128, 128], "pswarm")
        for _ in range(6):
            nc.tensor.matmul(pswarm, ident, ident, start=True, stop=True)
        wread = cpool.tile([128, 128], f32)
        nc.scalar.copy(wread, pswarm)

        nc.gpsimd.memset(feats[:, :, 384:385], 1.0)

        # ---------------- segment one-hots (vector) ----------------
        oh_t = dpool.tile([128, 8, 64], f16)
        for qc in range(8):
            nc.vector.tensor_scalar(out=oh_t[:, qc, :], in0=iot,
                                    scalar1=idst[:, qc:qc + 1], scalar2=None,
                                    op0=ALU.is_equal)
        gacc = dpool.tile([128, 5, 64], f16)
        for jc in range(5):
            nc.vector.tensor_scalar(out=gacc[:, jc, :], in0=iot,
                                    scalar1=cidt[:, jc, 0:1], scalar2=None,
                                    op0=ALU.is_equal)
            for corner in range(1, 4):
                nc.vector.scalar_tensor_tensor(
                    out=gacc[:, jc, :], in0=iot,
                    scalar=cidt[:, jc, corner:corner + 1],
                    in1=gacc[:, jc, :], op0=ALU.is_equal, op1=ALU.add)

        # ---------------- segment means avg' [64, 384] ----------------
        psums = ps.tile([64, 385], f32, tag="ps_sums", bufs=1)
        for qc in range(8):
            nc.tensor.matmul(psums, oh_t[:, qc, :], feats[:, qc, :],
                             start=(qc == 0), stop=(qc == 7))
        cnt4 = spool.tile([64, 1], f32, tag="cnt4")
        nc.vector.tensor_scalar(out=cnt4, in0=psums[:, 384:385], scalar1=1.0,
                                scalar2=4.0, op0=ALU.max, op1=ALU.mult)
        recip4 = spool.tile([64, 1], f32, tag="recip4")
        nc.vector.reciprocal(out=recip4, in_=cnt4)
        avg_t = dpool.tile([64, 384], f16)
        nc.vector.tensor_scalar_mul(avg_t, psums[:, 0:384], recip4[:, 0:1])

        # ---------------- G masks -> Gr [64, 640] ----------------
        gr_t = dpool.tile([64, 640], f16)
        for jc in range(5):
            ptr = PSG([64, 128], f"ptr{jc}", dt=f16)
            nc.tensor.transpose(ptr, gacc[:, jc, :], ident)
            nc.scalar.copy(gr_t[:, jc * 128:(jc + 1) * 128], ptr)

        # ---------------- sm (f8, scaled by S_SM) ----------------
        sm_pad = dpool.tile([128, 3, SMR, SMW], f8)
        nc.gpsimd.memset(sm_pad, 0.0)
        for mc in range(3):
            for nch in range(2):
                psm = MAIN([128, 320], f"psm{mc}{nch}")
                nc.tensor.matmul(psm, avg_t[:, mc * 128:(mc + 1) * 128],
                                 gr_t[:, nch * 320:(nch + 1) * 320],
                                 start=True, stop=True)
                nc.scalar.activation(
                    out=sm_pad[:, mc, nch * 10:(nch + 1) * 10, 1:33],
                    in_=psm.rearrange("p (r c) -> p r c", c=32),
                    func=AF.Copy, scale=S_SM)

        # ---------------- h convs (fp8 DoubleRow over kc pairs + taps) ------
        hps = []

        def h_conv(cv):
            hp = dpool.tile([128, HR, SMW], f8, tag=f"hpad{cv}", name=f"hpad{cv}")
            nc.gpsimd.memset(hp, 0.0)
            for nch in range(2):
                psh = MAIN([128, 9 * 32], f"psh{cv}{nch}")
                psh3 = psh.rearrange("p (r c) -> p r c", c=32)
                # kc-pair (0,1) DoubleRow per tap (k-tile delta = plane stride)
                for t, (dy, dx) in enumerate(TAPS):
                    r0 = nch * 9 + dy
                    mv = sm_pad[:, 0, r0:r0 + 9, dx:dx + 32]
                    mv2 = _pair_ap(mv, SMR * SMW)
                    nc.tensor.matmul(psh, ws01_t[:, cv, t, :, :], mv2,
                                     start=(t == 0), stop=False, perf_mode=DR)
                # kc=2: vertical tap-pairs (delta = row stride) + 3 singles
                for i, (ta, tb) in enumerate(VPAIRS):
                    dy, dx = TAPS[ta]
                    r0 = nch * 9 + dy
                    mv = sm_pad[:, 2, r0:r0 + 9, dx:dx + 32]
                    mv2 = _pair_ap(mv, SMW)
                    nc.tensor.matmul(psh, ws2_t[:, cv, 2 * i:2 * i + 2, :], mv2,
                                     start=False, stop=False, perf_mode=DR)
                for j, t in enumerate(VSINGLES):
                    dy, dx = TAPS[t]
                    r0 = nch * 9 + dy
                    nc.tensor.matmul(psh, ws2_t[:, cv, 6 + j, :],
                                     sm_pad[:, 2, r0:r0 + 9, dx:dx + 32],
                                     start=False, stop=(j == 2))
                # zero the out-of-image halo row, then relu -> f8 (scaled S_H)
                nc.vector.tensor_tensor(
                    out=psh3, in0=psh3,
                    in1=hmask_bc[:, nch * 9:(nch + 1) * 9, None].to_broadcast([128, 9, 32]),
                    op=ALU.mult)
                nc.scalar.activation(
                    out=hp[:, nch * 9:(nch + 1) * 9, 1:33], in_=psh3,
                    func=AF.Relu, bias=bs_t[:, cv:cv + 1],
                    scale=S_H / (S_SM * S_WS))
            return hp

        # fold conv: out [64, 512] from hp windows, stationary wsm9 cols
        def fold_conv(hp, cols, pstile):
            for i, (ta, tb) in enumerate(VPAIRS):
                dy, dx = TAPS[ta]
                mv2 = _pair_ap(hp[:, dy:dy + 16, dx:dx + 32], SMW)
                nc.tensor.matmul(pstile, wsm9_t[:, 2 * i:2 * i + 2, cols], mv2,
                                 start=(i == 0), stop=False, perf_mode=DR)
            for j, t in enumerate(VSINGLES):
                dy, dx = TAPS[t]
                nc.tensor.matmul(pstile, wsm9_t[:, 6 + j, cols],
                                 hp[:, dy:dy + 16, dx:dx + 32],
                                 start=False, stop=(j == 2))

        # ------------- LN0 partial stats (vector; x lands early now) -------
        bno0 = dpool.tile([128, 12, 6], f32)
        for kc in range(12):
            nc.vector.bn_stats(out=bno0[:, kc, :], in_=xt[:, kc, :])
        mv0 = spool.tile([128, 2], f32, tag="mv0")
        nc.vector.bn_aggr(out=mv0, in_=bno0)
        mvE0 = spool.tile([128, 2], f32, tag="mvE0")
        nc.vector.tensor_tensor(out=mvE0[:, 0:1], in0=mv0[:, 0:1],
                                in1=mv0[:, 0:1], op=ALU.mult)
        nc.vector.tensor_tensor(out=mvE0[:, 1:2], in0=mv0[:, 1:2],
                                in1=mvE0[:, 0:1], op=ALU.add)
        nc.vector.tensor_scalar(out=mvE0[:, 0:1], in0=mv0[:, 0:1],
                                scalar1=1.0, scalar2=None, op0=ALU.mult)

        h0p = h_conv(0)
        psC0 = PSC([64, 512], "psC0")
        fold_conv(h0p, slice(0, 64), psC0)
        psB0 = PSB([64, 512], "psB0")
        fold_conv(h0p, slice(64, 128), psB0)

        # partition-reduce (PE ones-matmul at an idle point) -> DRAM ->
        # AllReduce -> broadcast the raw sums back to all partitions
        def ln_stats(i, mvE, p_out, bc_tile):
            p_in = mvE.shape[0]
            pst = PSG([2, 1], f"pst{i}")
            nc.tensor.matmul(pst, mvE, ones_col[0:p_in, :],
                             start=True, stop=True)
            st_sb = spool.tile([2, 1], f32, tag=f"st_sb{i}")
            nc.vector.tensor_copy(out=st_sb, in_=pst)
            nc.gpsimd.dma_start(out=st_l[i][:], in_=st_sb[0:2, 0:1])
            nc.gpsimd.collective_compute(
                "AllReduce", ALU.add,
                replica_groups=[[0, 1], [2, 3], [4, 5], [6, 7]],
                ins=[st_l[i][:]], outs=[st_g[i][:]],
            )
            nc.gpsimd.dma_start(out=bc_tile,
                                in_=st_g[i][None, :].to_broadcast([p_out, 2]))

        sums0 = spool.tile([8, 2], f32, tag="sums0")

        # ---------------- conv_g + A0 pipeline ----------------
        gp1 = dpool.tile([128, 12, 512], f16)
        xg = dpool.tile([128, 12, 512], f16)
        psA0 = PSA([8, 512], "psA0")

        def conv_g_chunk(kc):
            psg = MAIN([128, 512], f"psg{kc}")
            for i, (ta, tb) in enumerate(VPAIRS):
                dy, dx = TAPS[ta]
                mv2 = _pair_ap(h0p[:, dy:dy + 16, dx:dx + 32], SMW)
                nc.tensor.matmul(psg, wg_t[:, kc, 2 * i:2 * i + 2, :], mv2,
                                 start=(i == 0), stop=False, perf_mode=DR)
            for j, t in enumerate(VSINGLES):
                dy, dx = TAPS[t]
                nc.tensor.matmul(psg, wg_t[:, kc, 6 + j, :],
                                 h0p[:, dy:dy + 16, dx:dx + 32],
                                 start=False, stop=(j == 2))
            nc.scalar.activation(out=gp1[:, kc, :], in_=psg, func=AF.Identity,
                                 bias=gb0_t[:, kc:kc + 1], scale=1.0 / (S_WG * S_H))
            nc.gpsimd.tensor_tensor(out=xg[:, kc, :], in0=xt[:, kc, :],
                                     in1=gp1[:, kc, :], op=ALU.mult)

        def a0_mm(kc):
            nc.tensor.matmul(psA0, w0t_t[:, kc, :], xg[:, kc, :],
                             start=(kc == 0), stop=(kc == 11))

        for kc in range(12):
            conv_g_chunk(kc)
            if kc >= 2:
                a0_mm(kc - 2)
            if kc == 2:
                # LN0 stats round-trip launches while conv_g still runs
                ln_stats(0, mvE0, 8, sums0)
        a0_mm(10)
        a0_mm(11)

        # ---------------- SPADE1/2 convs (all stat-independent) -------------
        # psB1/psC2/psB2 folds are emitted later (PE tail): they reuse PSUM
        # banks whose previous tenants die only at the z0/z1 combines.
        h1p = h_conv(1)
        psg1 = PSG([64, 512], "psg1")
        fold_conv(h1p, slice(128, 192), psg1)
        gp11 = spool.tile([8, 512], f16, tag="gp11")
        nc.scalar.activation(out=gp11, in_=psg1[0:8, :], func=AF.Identity,
                             bias=gb1_t, scale=1.0 / (S_WG * S_H))
        psC1 = PSC([64, 512], "psC1")
        fold_conv(h1p, slice(192, 256), psC1)
        h2p = h_conv(2)
        psg2 = PSG([64, 512], "psg2")
        fold_conv(h2p, slice(320, 384), psg2)
        gp12 = spool.tile([16, 512], f16, tag="gp12")
        nc.scalar.activation(out=gp12, in_=psg2[0:16, :], func=AF.Identity,
                             bias=gb2_t, scale=1.0 / (S_WG * S_H))

        # ---------------- stats math + combine helpers ----------------
        def ln_finish(i, sums, p, n_inv, work_tag):
            """sums [p,2] broadcast raw sums -> work [p,4]: mu, r, -mu*r."""
            w = spool.tile([p, 6], f32, tag=work_tag)
            nc.vector.tensor_scalar_mul(w[:, 0:2], sums, n_inv)   # mu, E
            nc.vector.tensor_tensor(out=w[:, 2:3], in0=w[:, 0:1],
                                    in1=w[:, 0:1], op=ALU.mult)   # mu^2
            nc.vector.scalar_tensor_tensor(out=w[:, 3:4], in0=w[:, 2:3],
                                           scalar=-1.0, in1=w[:, 1:2],
                                           op0=ALU.mult, op1=ALU.add)  # var
            nc.vector.tensor_scalar(out=w[:, 3:4], in0=w[:, 3:4],
                                    scalar1=1.0, scalar2=1e-12,
                                    op0=ALU.mult, op1=ALU.add)
            nc.vector.reciprocal_approx_fast(out=w[:, 4:5], in_=w[:, 3:4])
            nc.scalar.sqrt(w[:, 5:6], w[:, 4:5])                  # r
            nc.vector.tensor_tensor(out=w[:, 2:3], in0=w[:, 0:1],
                                    in1=w[:, 5:6], op=ALU.mult)   # mu*r
            nc.vector.tensor_scalar_mul(w[:, 3:4], w[:, 2:3], -1.0)  # -mu*r
            return w  # r = w[:,5:6], negmur = w[:,3:4]

        def combine(psA, psC, psB, w, Bc, bf, sCB, p, tag):
            """z = r*A + sCB*((-mu r)*Bv + Cv) + (bf + (-mu r)*Bc)."""
            bias_dyn = spool.tile([p, 1], f32, tag=f"bd{tag}")
            nc.vector.scalar_tensor_tensor(out=bias_dyn, in0=Bc,
                                           scalar=w[:, 3:4], in1=bf,
                                           op0=ALU.mult, op1=ALU.add)
            tb = dpool.tile([p, 512], f32, tag=f"tb{tag}")
            nc.vector.tensor_scalar_mul(tb, psB, w[:, 3:4])
            t1 = dpool.tile([p, 512], f32, tag=f"t1{tag}")
            nc.vector.tensor_tensor(out=t1, in0=tb, in1=psC, op=ALU.add)
            t2 = dpool.tile([p, 512], f32, tag=f"t2{tag}")
            nc.vector.tensor_scalar(out=t2, in0=t1, scalar1=sCB,
                                    scalar2=bias_dyn, op0=ALU.mult, op1=ALU.add)
            z = dpool.tile([p, 512], f32, tag=f"z{tag}")
            nc.vector.scalar_tensor_tensor(out=z, in0=psA, scalar=w[:, 5:6],
                                           in1=t2, op0=ALU.mult, op1=ALU.add)
            return z

        # ---------------- layer 0 tail ----------------
        w0 = ln_finish(0, sums0, 8, 1.0 / 256.0, "w0")
        z0 = combine(psA0, psC0[0:8, :], psB0[0:8, :], w0, B0c_t, b0f_t,
                     1.0 / (S_F0 * S_H), 8, "0")
        out0 = dpool.tile([8, 512], f32)
        _softplus(nc, dpool, z0, out0, 8, "0")

        # og1 first (only needs out0), then LN1 stats, then PE tail folds
        og1 = spool.tile([8, 512], f16, tag="og1")
        nc.vector.tensor_tensor(out=og1, in0=out0, in1=gp11, op=ALU.mult)
        bno1 = spool.tile([8, 1, 6], f32, tag="bno1")
        nc.vector.bn_stats(out=bno1[:, 0, :], in_=out0)
        mv1 = spool.tile([8, 2], f32, tag="mv1")
        nc.vector.bn_aggr(out=mv1, in_=bno1)
        mvE1 = spool.tile([8, 2], f32, tag="mvE1")
        nc.vector.tensor_tensor(out=mvE1[:, 0:1], in0=mv1[:, 0:1],
                                in1=mv1[:, 0:1], op=ALU.mult)
        nc.vector.tensor_tensor(out=mvE1[:, 1:2], in0=mv1[:, 1:2],
                                in1=mvE1[:, 0:1], op=ALU.add)
        nc.vector.tensor_scalar(out=mvE1[:, 0:1], in0=mv1[:, 0:1],
                                scalar1=1.0, scalar2=None, op0=ALU.mult)

        psB1 = PSB([64, 512], "psB1")
        fold_conv(h1p, slice(256, 320), psB1)
        psC2 = PSC([64, 512], "psC2")
        fold_conv(h2p, slice(384, 448), psC2)
        psA1 = PSA([16, 512], "psA1")
        nc.tensor.matmul(psA1, w1t_t, og1, start=True, stop=True)
        sums1 = spool.tile([16, 2], f32, tag="sums1")
        ln_stats(1, mvE1, 16, sums1)

        # ---------------- layer 1 tail ----------------
        w1 = ln_finish(1, sums1, 16, 1.0 / 16.0, "w1")
        z1 = combine(psA1, psC1[0:16, :], psB1[0:16, :], w1, B1c_t, b1f_t,
                     1.0 / (S_F12 * S_H), 16, "1")
        out1 = dpool.tile([16, 512], f32)
        _softplus(nc, dpool, z1, out1, 16, "1")

        # og2 first, then LN2 stats; PE: psB2 fold, A2, pst2
        og2 = spool.tile([16, 512], f16, tag="og2")
        nc.vector.tensor_tensor(out=og2, in0=out1, in1=gp12, op=ALU.mult)
        bno2 = spool.tile([16, 1, 6], f32, tag="bno2")
        nc.vector.bn_stats(out=bno2[:, 0, :], in_=out1)
        mv2 = spool.tile([16, 2], f32, tag="mv2")
        nc.vector.bn_aggr(out=mv2, in_=bno2)
        mvE2 = spool.tile([16, 2], f32, tag="mvE2")
        nc.vector.tensor_tensor(out=mvE2[:, 0:1], in0=mv2[:, 0:1],
                                in1=mv2[:, 0:1], op=ALU.mult)
        nc.vector.tensor_tensor(out=mvE2[:, 1:2], in0=mv2[:, 1:2],
                                in1=mvE2[:, 0:1], op=ALU.add)
        nc.vector.tensor_scalar(out=mvE2[:, 0:1], in0=mv2[:, 0:1],
                                scalar1=1.0, scalar2=None, op0=ALU.mult)

        psB2 = PSB([64, 512], "psB2")
        fold_conv(h2p, slice(448, 512), psB2)
        psA2 = PSA([1, 512], "psA2")
        nc.tensor.matmul(psA2, w2t_t, og2, start=True, stop=True)
        sums2 = spool.tile([1, 2], f32, tag="sums2")
        ln_stats(2, mvE2, 1, sums2)

        # ---------------- layer 2 tail -> output ----------------
        w2 = ln_finish(2, sums2, 1, 1.0 / 32.0, "w2")
        z2 = combine(psA2, psC2[0:1, :], psB2[0:1, :], w2, B2c_t, b2f_t,
                     1.0 / (S_F12 * S_H), 1, "2")
        final = dpool.tile([1, 512], f32)
        _softplus(nc, dpool, z2, final, 1, "2")
        nc.scalar.dma_start(out=d_out[:], in_=final[0:1, :])

    nc.compile()
    return nc


def _host_prep(inputs):
    """Build per-core in_maps (host work: slicing, layout, small weight folds)."""
    x_main = np.asarray(inputs["x_main"], np.float32)
    f_sem = np.asarray(inputs["f_sem"], np.float32)
    seg = np.asarray(inputs["seg_mask"])
    f8np = ml_dtypes.float8_e4m3

    def lhsT9(w):  # [O, I, 3, 3] -> [I, 9, O]
        return np.ascontiguousarray(w.transpose(1, 2, 3, 0).reshape(w.shape[1], 9, w.shape[0]))

    ws_stack = np.stack([inputs["s0_ws"], inputs["s1_ws"], inputs["s2_ws"]])  # [3,128,384,3,3]
    ws_r = ws_stack.reshape(3, 128, 3, 128, 3, 3)          # cv, o, kc, i, ky, kx
    ws_full = (ws_r.transpose(3, 0, 2, 4, 5, 1)
               .reshape(128, 3, 3, 9, 128) * S_WS)         # [i, cv, kc, tap, o]
    WS01 = np.ascontiguousarray(
        ws_full[:, :, 0:2].transpose(0, 1, 3, 2, 4)).astype(f8np)  # [128,3,9,2,128]
    WS2 = np.ascontiguousarray(ws_full[:, :, 2][:, :, TAP_ORDER]).astype(f8np)
    wg0 = np.asarray(inputs["s0_wg"], np.float32)          # [1536, 128, 3, 3]
    WG = np.ascontiguousarray(
        (wg0.reshape(12, 128, 128, 3, 3).transpose(2, 0, 3, 4, 1)
         .reshape(128, 12, 9, 128) * S_WG)[:, :, TAP_ORDER]).astype(f8np)
    wf0 = np.einsum("oc,cikl->oikl", np.asarray(inputs["conv0_w"], np.float64),
                    np.asarray(inputs["s0_wb"], np.float64))
    wb0 = np.einsum("oc,cikl->oikl", np.asarray(inputs["conv0_w"], np.float64),
                    np.asarray(inputs["s0_wg"], np.float64))
    wf1 = np.einsum("oc,cikl->oikl", np.asarray(inputs["conv1_w"], np.float64),
                    np.asarray(inputs["s1_wb"], np.float64))
    wb1 = np.einsum("oc,cikl->oikl", np.asarray(inputs["conv1_w"], np.float64),
                    np.asarray(inputs["s1_wg"], np.float64))
    wf2 = np.einsum("oc,cikl->oikl", np.asarray(inputs["conv2_w"], np.float64),
                    np.asarray(inputs["s2_wb"], np.float64))
    wb2 = np.einsum("oc,cikl->oikl", np.asarray(inputs["conv2_w"], np.float64),
                    np.asarray(inputs["s2_wg"], np.float64))
    WSM9 = np.zeros((128, 9, 512), np.float64)
    WSM9[:, :, 0:8] = lhsT9(wf0) * S_F0
    WSM9[:, :, 64:72] = lhsT9(wb0) * S_F0
    WSM9[:, :, 128:136] = lhsT9(np.asarray(inputs["s1_wg"], np.float64)) * S_WG
    WSM9[:, :, 192:208] = lhsT9(wf1) * S_F12
    WSM9[:, :, 256:272] = lhsT9(wb1) * S_F12
    WSM9[:, :, 320:336] = lhsT9(np.asarray(inputs["s2_wg"], np.float64)) * S_WG
    WSM9[:, :, 384:385] = lhsT9(wf2) * S_F12
    WSM9[:, :, 448:449] = lhsT9(wb2) * S_F12
    WSM9 = WSM9[:, TAP_ORDER].astype(f8np)  # [128, 9, 512], tap-reordered
    W0T = np.ascontiguousarray(np.asarray(inputs["conv0_w"], np.float32).T
                               .reshape(12, 128, 8).transpose(1, 0, 2)).astype(np.float16)
    WSM = np.concatenate([
        np.asarray(inputs["conv1_w"], np.float32).T.reshape(-1),
        np.asarray(inputs["conv2_w"], np.float32).T.reshape(-1)]).astype(np.float16)  # [144]
    BSGB = np.concatenate([
        np.stack([inputs["s0_bs"], inputs["s1_bs"], inputs["s2_bs"]]).T * S_H,  # [128,3]
        (1.0 + np.asarray(inputs["s0_bg"], np.float32)).reshape(12, 128).T,     # [128,12]
    ], axis=1).astype(np.float32)                                               # [128,15]
    c0w = np.asarray(inputs["conv0_w"], np.float64)
    c1w = np.asarray(inputs["conv1_w"], np.float64)
    c2w = np.asarray(inputs["conv2_w"], np.float64)
    BIASV = np.zeros((16, 8), np.float32)
    BIASV[0:8, 0] = 1.0 + np.asarray(inputs["s1_bg"], np.float64)
    BIASV[0:16, 1] = 1.0 + np.asarray(inputs["s2_bg"], np.float64)
    BIASV[0:8, 2] = (np.asarray(inputs["b0"], np.float64)
                     + c0w @ np.asarray(inputs["s0_bb"], np.float64))
    BIASV[0:16, 3] = (np.asarray(inputs["b1"], np.float64)
                      + c1w @ np.asarray(inputs["s1_bb"], np.float64))
    BIASV[0:1, 4] = (np.asarray(inputs["b2"], np.float64)
                     + c2w @ np.asarray(inputs["s2_bb"], np.float64))
    BIASV[0:8, 5] = c0w @ (1.0 + np.asarray(inputs["s0_bg"], np.float64))
    BIASV[0:16, 6] = c1w @ (1.0 + np.asarray(inputs["s1_bg"], np.float64))
    BIASV[0:1, 7] = c2w @ (1.0 + np.asarray(inputs["s2_bg"], np.float64))

    shared = dict(ws01=WS01, ws2=WS2, wg=WG, wsm9=WSM9, w0t=W0T,
                  wsm=WSM, bsgb=BSGB, biasv=BIASV)

    in_maps = []
    for core in range(8):
        k, h = core // 2, core % 2
        r0 = HROWS * h
        X = np.ascontiguousarray(
            x_main[k, :, r0:r0 + HROWS, :].reshape(12, 128, 512).transpose(1, 0, 2)
        ).astype(np.float16)
        FT = np.ascontiguousarray(
            f_sem[k].reshape(384, NPOS).T.reshape(8, 128, 384).transpose(1, 0, 2)
        ).astype(np.float16)
        ids_flat = seg[k, ::14, ::14].astype(np.float32).reshape(NPOS)
        IDS = np.ascontiguousarray(ids_flat.reshape(8, 128).T)
        rows = np.arange(r0 - 2, r0 + HROWS + 2)          # 20 sm rows
        valid = (rows >= 0) & (rows < Hp)
        rcl = np.clip(rows, 0, Hp - 1)
        cid = np.empty((SMR, Wp, 4), np.float32)
        cols = np.arange(Wp)
        for t, (dy, dx) in enumerate([(0, 0), (0, 1), (1, 0), (1, 1)]):
            v = seg[k][np.ix_(14 * rcl + 6 + dy, 14 * cols + 6 + dx)].astype(np.float32)
            v[~valid, :] = -1.0
            cid[:, :, t] = v
        CID = np.ascontiguousarray(cid.reshape(5, 128, 4).transpose(1, 0, 2))
        hrows = np.arange(r0 - 1, r0 + HROWS + 1)
        HM = ((hrows >= 0) & (hrows < Hp)).astype(np.float32)
        in_maps.append(dict(shared, x=X, ft=FT, ids=IDS, cid=CID, hmask=HM))
    return in_maps


def kernel(**inputs):
    global _BUILT, LAST_RESULTS
    if _BUILT is None:
        _BUILT = _build_nc()
    nc = _BUILT
    in_maps = _host_prep(inputs)
    trace = bool(os.environ.get("BASS_TRACE"))
    res = run_bass_kernel_spmd(nc, in_maps, list(range(8)), trace=trace)
    LAST_RESULTS = res
    out = np.empty((B, 1, Hp, Wp), np.float32)
    for core in range(8):
        k, h = core // 2, core % 2
        out[k, 0, HROWS * h:HROWS * (h + 1), :] = \
            res.results[core]["out_half"].reshape(HROWS, Wp)
    return out
